# revision 8
# baseline (speedup 1.0000x reference)
"""Trainium2 Bass kernel for a single attention head.

Reference math (per batch b):
    q = emb @ Wq.T + bq ; k = emb @ Wk.T + bk ; v = emb @ Wv.T + bv
    attn = softmax((q @ k.T) / sqrt(768), axis=-1)
    out  = attn @ v

Sharding: pure data-parallel over batch. B=8 batches onto 8 NeuronCores,
one batch per core, no collectives.

Device-side layout strategy (per core):
  - emb arrives pre-transposed from the host as embT (768, 2048) bf16, one
    DMA per 512-column chunk ([128, 6, 512] tiles), so the E=768 contraction
    dim sits on SBUF partitions and HWDGE dispatch stays off the critical
    path.
  - Q and K projections are FUSED: one stationary [WqT|WkT] (768, 128) block
    computes Q^T (PSUM rows 0:64) and K^T (rows 64:128) in a single moving
    pass over each embT chunk.
  - bk is dropped: softmax over k is invariant to it.
  - Q^T/K^T are evacuated to fp8e4m3 in the DoubleRow-folded layout
    [32, 2, S] (plane j holds head dims j*32:(j+1)*32) via cross-
    partition-base DVE/Pool copies; the score matmuls then run in fp8
    DoubleRow perf mode at 0.5 cycles/row (2x bf16). Adds ~1% rel err
    (measured 1.34e-2 total vs the 2e-2 gate).
  - scores are computed transposed, S^T[k, q]; exp(S^T) (ACT, scale folded
    in) feeds the AV stage as bf16. No max-subtraction: scores*scale has
    std ~0.3, exp is safe in f32.
  - AV runs FLIPPED: P^T 128x128 slices are the stationary operand and
    V' (64 cols of V + an all-ones column for the softmax denominator Z)
    is the 65-column moving operand, accumulated over the 16 k-tiles.
    The output lands q-on-partitions, so no output transposes are needed;
    the final divide folds Z out directly from PSUM.
"""

import sys

import numpy as np

try:
    import concourse.bass as bass  # noqa: F401
except ImportError:  # pragma: no cover
    sys.path.insert(0, "/opt/trn_rl_repo")

from contextlib import ExitStack

import ml_dtypes

import concourse.bass as bass
import concourse.tile as tile
from concourse import mybir
from concourse.bass_utils import run_bass_kernel_spmd
from concourse.masks import make_identity

S = 2048  # sequence length
E = 768  # embedding dim
D = 64  # inner (head) dim
NCORES = 8
SCALE = float(1.0 / np.sqrt(np.float32(768.0)))

F32 = mybir.dt.float32
BF16 = mybir.dt.bfloat16
F8 = mybir.dt.float8e4
AF = mybir.ActivationFunctionType
DR = mybir.MatmulPerfMode.DoubleRow

QB = 512  # q block (one PSUM bank of f32)
NQB = S // QB  # 4 q blocks
NKT = S // 128  # 16 k tiles of 128
NKP = NKT // 2  # 8 k tile pairs


def split_multi_waits(nc: bass.Bass) -> int:
    """This toolchain's walrus encodes at most ONE semaphore wait per
    instruction ("Too many sync wait commands" otherwise). Tile freely emits
    multi-wait instructions, so hoist all but the last wait onto preceding
    same-engine NoOps — sequencer waits gate dispatch, so semantics are
    identical."""
    nsplit = 0
    for f in nc.m.functions:
        for bb in f.blocks:
            out = []
            changed = False
            for inst in bb.instructions:
                si = getattr(inst, "sync_info", None)
                if si is not None and len(si.on_wait) > 1:
                    waits = list(si.on_wait)
                    for w in waits[:-1]:
                        out.append(
                            mybir.InstNoOp(
                                name=nc.get_next_instruction_name(),
                                engine=inst.engine,
                                bass_nofuse=True,
                                sync_info=mybir.SyncInfo(on_wait=[w], on_update=[]),
                            )
                        )
                    inst.sync_info = mybir.SyncInfo(
                        on_wait=[waits[-1]], on_update=list(si.on_update)
                    )
                    changed = True
                    nsplit += 1
                out.append(inst)
            if changed:
                bb.instructions = out
    return nsplit


def build_nc(variant: str = "full", reps: int = 1) -> bass.Bass:
    do_proj = variant in ("full", "proj", "projattn")
    do_attn = variant in ("full", "projattn")
    do_out = variant == "full"
    nc = bass.Bass()

    embT_h = nc.declare_dram_parameter("embT", [E, S], BF16, isOutput=False)
    wts_h = nc.declare_dram_parameter("wts", [E, 192], BF16, isOutput=False)
    bias_h = nc.declare_dram_parameter("biases", [128, 2], F32, isOutput=False)
    out_h = nc.declare_dram_parameter("out", [S, D], F32, isOutput=True)

    with tile.TileContext(nc) as tc, ExitStack() as ctx:
        const = ctx.enter_context(tc.tile_pool(name="const", bufs=1))
        sb = ctx.enter_context(tc.tile_pool(name="sb", bufs=1))

        # ---- constants / small inputs ----
        bias_sb = const.tile([128, 2], F32, tag="bias")
        nc.sync.dma_start(out=bias_sb[:], in_=bias_h[:])
        # weights: one DMA, (768, 192) -> (128, 6, 192): [WqT|WkT|WvT]
        wts_all = const.tile([128, 6, 192], BF16, tag="wts")
        nc.sync.dma_start(
            out=wts_all[:], in_=wts_h[:].rearrange("(c p) w -> p c w", p=128)
        )
        ident_bf = const.tile([128, 128], BF16, tag="idbf")
        make_identity(nc, ident_bf[:])

        # warm the ACT exp table set while DMAs run
        warm = const.tile([128, 8], F32, tag="warm")
        nc.gpsimd.memset(warm[:], 0.0)
        nc.scalar.activation(warm[:], warm[:], AF.Exp)

        # ---- persistent SBUF ----
        # Q^T / K^T in fp8 DoubleRow-folded layout: [32, 2, S], plane j holds
        # head dims j*32:(j+1)*32.
        qf_sb = sb.tile([32, 2, S], F8, tag="qf")
        kf_sb = sb.tile([32, 2, S], F8, tag="kf")
        vt_sb = sb.tile([64, S], BF16, tag="vt")
        # V' tiles: (k-tile, 65) with col 64 == 1.0 (softmax denominator)
        vv_sb = sb.tile([128, NKT, D + 1], BF16, tag="vv")
        nc.gpsimd.memset(vv_sb[:, :, D : D + 1], 1.0)
        out_sb = sb.tile([128, NKT, D], F32, tag="outsb")

        embT_sb = [None] * NQB

        def dma_embT_chunk(n):
            # one DMA per 512-col chunk: (768, 512) -> (128, 6, 512)
            t = sb.tile([128, 6, QB], BF16, tag=f"embT_{n}")
            nc.sync.dma_start(
                out=t[:],
                in_=embT_h[:, n * QB : (n + 1) * QB].rearrange(
                    "(c p) w -> p c w", p=128
                ),
            )
            embT_sb[n] = t

        with (
            tc.tile_pool(name="psA", bufs=1, space="PSUM") as psA,
            tc.tile_pool(name="psT", bufs=1, space="PSUM") as psT,
            tc.tile_pool(name="psO", bufs=2, space="PSUM") as psO,
            tc.tile_pool(name="psS", bufs=2, space="PSUM") as psS,
            tc.tile_pool(name="ptp", bufs=16) as ptp,
            tc.tile_pool(name="rcp", bufs=4) as rcp,
        ):
            oacc_tiles = {}
            pt_tiles = {}

            def proj_qk_chunk(n):
                qs = slice(n * QB, (n + 1) * QB)
                ps = psA.tile([128, QB], F32, tag="proj")
                for c in range(6):
                    nc.tensor.matmul(
                        ps[:, :],
                        lhsT=wts_all[:, c, 0:128],
                        rhs=embT_sb[n][:, c, :],
                        start=(c == 0),
                        stop=(c == 5),
                    )
                # evacuate + fold to fp8 DoubleRow layout (cross-base copies)
                nc.vector.tensor_scalar_add(
                    qf_sb[:, 0, qs], ps[0:32, :], bias_sb[0:32, 0:1]
                )
                nc.vector.tensor_scalar_add(
                    qf_sb[:, 1, qs], ps[32:64, :], bias_sb[32:64, 0:1]
                )
                nc.vector.tensor_copy(out=kf_sb[:, 0, qs], in_=ps[64:96, :])
                nc.vector.tensor_copy(out=kf_sb[:, 1, qs], in_=ps[96:128, :])

            def proj_v_chunk(n):
                qs = slice(n * QB, (n + 1) * QB)
                ps = psA.tile([128, QB], F32, tag="proj")
                for c in range(6):
                    nc.tensor.matmul(
                        ps[0:64, :],
                        lhsT=wts_all[:, c, 128:192],
                        rhs=embT_sb[n][:, c, :],
                        start=(c == 0),
                        stop=(c == 5),
                    )
                nc.vector.tensor_scalar_add(
                    vt_sb[0:64, qs], ps[0:64, :], bias_sb[0:64, 1:2]
                )
                # V^T chunk -> 4 V' tiles (128, 64) via PE transpose
                vtp = psT.tile([128, 256], BF16, tag="vtp")
                for jj in range(4):
                    j = 4 * n + jj
                    nc.tensor.transpose(
                        vtp[:, jj * 64 : (jj + 1) * 64],
                        vt_sb[0:64, j * 128 : (j + 1) * 128],
                        ident_bf[0:64, 0:64],
                    )
                nc.vector.tensor_copy(
                    out=vv_sb[:, 4 * n : 4 * n + 4, 0:D],
                    in_=vtp[:].rearrange("p (j d) -> p j d", j=4),
                )

            def attn_pair(n, j):
                qs = slice(n * QB, (n + 1) * QB)
                sc = psS.tile([128, 1024], F32, tag="sc")
                # S^T tiles for k-tiles 2j and 2j+1, fp8 DoubleRow (0.5 cyc/row)
                for h in range(2):
                    kt = 2 * j + h
                    nc.tensor.matmul(
                        sc[:, h * 512 : (h + 1) * 512],
                        lhsT=kf_sb[:, :, kt * 128 : (kt + 1) * 128],
                        rhs=qf_sb[:, :, qs],
                        start=True,
                        stop=True,
                        perf_mode=DR,
                    )
                pt = ptp.tile([128, 1024], BF16, tag="pt")
                nc.scalar.activation(pt[:], sc[:], AF.Exp, scale=SCALE)
                pt_tiles[(n, j)] = pt

            def av_block(n):
                # flipped AV: P^T 128x128 slices stationary, V' 65-col moving.
                # sub is the OUTER loop: PSUM accumulation groups sharing one
                # bank must not interleave (hardware corrupts them otherwise).
                oacc = psO.tile([128, 4, D + 1], F32, tag="oacc", name=f"oacc{rep}_{n}")
                oacc_tiles[n] = oacc
                for sub in range(4):
                    for j in range(NKP):
                        pt = pt_tiles[(n, j)]
                        for h in range(2):
                            kt = 2 * j + h
                            nc.tensor.matmul(
                                oacc[:, sub, :],
                                lhsT=pt[
                                    :, h * 512 + sub * 128 : h * 512 + (sub + 1) * 128
                                ],
                                rhs=vv_sb[:, kt, :],
                                start=(j == 0 and h == 0),
                                stop=(j == NKP - 1 and h == 1),
                                skip_group_check=True,
                            )

            def out_block(n):
                oacc = oacc_tiles[n]
                for sub in range(4):
                    t = n * 4 + sub
                    rc = rcp.tile([128, 1], F32, tag="rc")
                    nc.vector.reciprocal(rc[:], oacc[:, sub, D : D + 1])
                    nc.vector.tensor_scalar_mul(
                        out_sb[:, t, :], oacc[:, sub, 0:D], rc[:, 0:1]
                    )
                qs = slice(n * QB, (n + 1) * QB)
                nc.sync.dma_start(
                    out=out_h[qs, :].rearrange("(t p) i -> p t i", p=128),
                    in_=out_sb[:, n * 4 : (n + 1) * 4, :],
                )

            # ---- software-pipelined emission, paced by chunk DMA arrival.
            # reps > 1 repeats the whole computation for benchmarking.
            for rep in range(reps):
                for n in range(NQB):
                    dma_embT_chunk(n)
                if rep == 0 and do_attn:
                    # PE warm-up during the DMA lead-in: dummy matmuls ramp the
                    # HAM clock gate (1.2 -> 2.4 GHz) before the first real
                    # projection matmul.
                    wmm = psS.tile([128, 1024], F32, tag="sc", name="warmmm")
                    for i in range(12):
                        nc.tensor.matmul(
                            wmm[:, (i % 2) * 512 : (i % 2) * 512 + 128],
                            lhsT=ident_bf[:, 0:128],
                            rhs=ident_bf[:, 0:128],
                            start=True,
                            stop=True,
                        )
                if do_proj:
                    proj_qk_chunk(0)
                    proj_v_chunk(0)
                if do_attn:
                    attn_pair(0, 0)
                    attn_pair(0, 1)
                if do_proj:
                    proj_qk_chunk(1)
                if do_attn:
                    attn_pair(1, 0)
                    attn_pair(1, 1)
                if do_proj:
                    proj_v_chunk(1)
                if do_attn:
                    for n in (0, 1):
                        attn_pair(n, 2)
                        attn_pair(n, 3)
                if do_proj:
                    proj_qk_chunk(2)
                    proj_v_chunk(2)
                if do_attn:
                    for n in (0, 1):
                        attn_pair(n, 4)
                        attn_pair(n, 5)
                if do_proj:
                    proj_qk_chunk(3)
                    proj_v_chunk(3)
                if do_attn:
                    for n in (0, 1):
                        attn_pair(n, 6)
                        attn_pair(n, 7)
                    av_block(0)
                    if do_out:
                        out_block(0)
                    attn_pair(2, 0)
                    attn_pair(2, 1)
                    av_block(1)
                    if do_out:
                        out_block(1)
                    for j in range(2, NKP):
                        attn_pair(2, j)
                    for j in range(NKP):
                        attn_pair(3, j)
                    av_block(2)
                    if do_out:
                        out_block(2)
                    av_block(3)
                    if do_out:
                        out_block(3)
            if not do_out:
                nc.gpsimd.memset(out_sb[:, 0:1, :], 0.0)
                nc.sync.dma_start(
                    out=out_h[:].rearrange("(t p) i -> p t i", p=128),
                    in_=out_sb[:],
                )

    split_multi_waits(nc)
    return nc


_NC_CACHE = None


def _get_nc():
    global _NC_CACHE
    if _NC_CACHE is None:
        _NC_CACHE = build_nc()
    return _NC_CACHE


def make_in_maps(emb_input, Wq, bq, Wk, bk, Wv, bv):
    bf16 = ml_dtypes.bfloat16
    WqT = np.ascontiguousarray(Wq.T).astype(bf16)  # (768, 64)
    WkT = np.ascontiguousarray(Wk.T).astype(bf16)
    WvT = np.ascontiguousarray(Wv.T).astype(bf16)
    wts = np.ascontiguousarray(
        np.concatenate([WqT, WkT, WvT], axis=1)
    )  # (768, 192)
    biases = np.zeros((128, 2), np.float32)
    biases[0:64, 0] = bq
    biases[0:64, 1] = bv
    in_maps = []
    for i in range(NCORES):
        embT = np.ascontiguousarray(emb_input[i].T).astype(bf16)  # (768, 2048)
        in_maps.append({"embT": embT, "wts": wts, "biases": biases})
    return in_maps


def run(emb_input, Wq, bq, Wk, bk, Wv, bv, trace=False):
    nc = _get_nc()
    in_maps = make_in_maps(emb_input, Wq, bq, Wk, bk, Wv, bv)
    res = run_bass_kernel_spmd(nc, in_maps, core_ids=list(range(NCORES)), trace=trace)
    out = np.stack([res.results[i]["out"] for i in range(NCORES)], axis=0)
    return out.astype(np.float32), res


def kernel(emb_input, Wq, bq, Wk, bk, Wv, bv):
    out, _ = run(emb_input, Wq, bq, Wk, bk, Wv, bv, trace=False)
    return out


# revision 13
# speedup vs baseline: 1.0754x; 1.0754x over previous
"""Trainium2 Bass kernel for a single attention head.

Reference math (per batch b):
    q = emb @ Wq.T + bq ; k = emb @ Wk.T + bk ; v = emb @ Wv.T + bv
    attn = softmax((q @ k.T) / sqrt(768), axis=-1)
    out  = attn @ v

Sharding: pure data-parallel over batch. B=8 batches onto 8 NeuronCores,
one batch per core, no collectives.

Device-side layout strategy (per core):
  - emb arrives pre-transposed from the host as embT (768, 2048) bf16, one
    DMA per 512-column chunk ([128, 6, 512] tiles), so the E=768 contraction
    dim sits on SBUF partitions and HWDGE dispatch stays off the critical
    path.
  - Q and K projections are FUSED: one stationary [WqT|WkT] (768, 128) block
    computes Q^T (PSUM rows 0:64) and K^T (rows 64:128) in a single moving
    pass over each embT chunk.
  - bk is dropped: softmax over k is invariant to it.
  - Q^T/K^T are evacuated to fp8e4m3 in the DoubleRow-folded layout
    [32, 2, S] (plane j holds head dims j*32:(j+1)*32) via cross-
    partition-base DVE/Pool copies; the score matmuls then run in fp8
    DoubleRow perf mode at 0.5 cycles/row (2x bf16). Adds ~1% rel err
    (measured 1.34e-2 total vs the 2e-2 gate).
  - scores are computed transposed, S^T[k, q]; exp(S^T) (ACT, scale folded
    in) feeds the AV stage as bf16. No max-subtraction: scores*scale has
    std ~0.3, exp is safe in f32.
  - AV runs FLIPPED: P^T 128x128 slices are the stationary operand and
    V' (64 cols of V + an all-ones column for the softmax denominator Z)
    is the 65-column moving operand, accumulated over the 16 k-tiles.
    The output lands q-on-partitions, so no output transposes are needed;
    the final divide folds Z out directly from PSUM.
"""

import sys

import numpy as np

try:
    import concourse.bass as bass  # noqa: F401
except ImportError:  # pragma: no cover
    sys.path.insert(0, "/opt/trn_rl_repo")

from contextlib import ExitStack

import ml_dtypes

import concourse.bass as bass
import concourse.tile as tile
from concourse import mybir
from concourse.bass_utils import run_bass_kernel_spmd
from concourse.masks import make_identity

S = 2048  # sequence length
E = 768  # embedding dim
D = 64  # inner (head) dim
NCORES = 8
SCALE = float(1.0 / np.sqrt(np.float32(768.0)))

F32 = mybir.dt.float32
BF16 = mybir.dt.bfloat16
F8 = mybir.dt.float8e4
AF = mybir.ActivationFunctionType
DR = mybir.MatmulPerfMode.DoubleRow

QB = 512  # q block (one PSUM bank of f32)
NQB = S // QB  # 4 q blocks
NKT = S // 128  # 16 k tiles of 128
NKP = NKT // 2  # 8 k tile pairs


def split_multi_waits(nc: bass.Bass) -> int:
    """This toolchain's walrus encodes at most ONE semaphore wait per
    instruction ("Too many sync wait commands" otherwise). Tile freely emits
    multi-wait instructions, so hoist all but the last wait onto preceding
    same-engine NoOps — sequencer waits gate dispatch, so semantics are
    identical."""
    nsplit = 0
    for f in nc.m.functions:
        for bb in f.blocks:
            out = []
            changed = False
            for inst in bb.instructions:
                si = getattr(inst, "sync_info", None)
                if si is not None and len(si.on_wait) > 1:
                    waits = list(si.on_wait)
                    for w in waits[:-1]:
                        out.append(
                            mybir.InstNoOp(
                                name=nc.get_next_instruction_name(),
                                engine=inst.engine,
                                bass_nofuse=True,
                                sync_info=mybir.SyncInfo(on_wait=[w], on_update=[]),
                            )
                        )
                    inst.sync_info = mybir.SyncInfo(
                        on_wait=[waits[-1]], on_update=list(si.on_update)
                    )
                    changed = True
                    nsplit += 1
                out.append(inst)
            if changed:
                bb.instructions = out
    return nsplit


def build_nc(variant: str = "full", reps: int = 1) -> bass.Bass:
    do_proj = variant in ("full", "proj", "projattn")
    do_attn = variant in ("full", "projattn")
    do_out = variant == "full"
    nc = bass.Bass()

    embT_h = nc.declare_dram_parameter("embT", [E, S], BF16, isOutput=False)
    wts_h = nc.declare_dram_parameter("wts", [E, 192], BF16, isOutput=False)
    bias_h = nc.declare_dram_parameter("biases", [128, 2], F32, isOutput=False)
    out_h = nc.declare_dram_parameter("out", [S, D], F32, isOutput=True)

    with tile.TileContext(nc) as tc, ExitStack() as ctx:
        const = ctx.enter_context(tc.tile_pool(name="const", bufs=1))
        sb = ctx.enter_context(tc.tile_pool(name="sb", bufs=1))

        # ---- constants / small inputs ----
        # weights first: one DMA, (768, 192) -> (128, 6, 192): [WqT|WkT|WvT]
        wts_all = const.tile([128, 6, 192], BF16, tag="wts")
        nc.sync.dma_start(
            out=wts_all[:], in_=wts_h[:].rearrange("(c p) w -> p c w", p=128)
        )
        bias_sb = const.tile([128, 2], F32, tag="bias")
        ident_bf = const.tile([128, 128], BF16, tag="idbf")
        make_identity(nc, ident_bf[:])

        # warm the ACT exp table set while DMAs run
        warm = const.tile([128, 8], F32, tag="warm")
        nc.gpsimd.memset(warm[:], 0.0)
        nc.scalar.activation(warm[:], warm[:], AF.Exp)

        # ---- persistent SBUF ----
        # Q^T / K^T in fp8 DoubleRow-folded layout: [32, 2, S], plane j holds
        # head dims j*32:(j+1)*32.
        qf_sb = sb.tile([32, 2, S], F8, tag="qf")
        kf_sb = sb.tile([32, 2, S], F8, tag="kf")
        vt_sb = sb.tile([64, S], BF16, tag="vt")
        # V' tiles: (k-tile, 65) with col 64 == 1.0 (softmax denominator)
        vv_sb = sb.tile([128, NKT, D + 1], BF16, tag="vv")
        nc.gpsimd.memset(vv_sb[:, :, D : D + 1], 1.0)
        out_sb = sb.tile([128, NKT, D], F32, tag="outsb")

        embT_sb = [None] * NQB

        def dma_embT_chunk(n):
            # two DMAs per 512-col chunk (e-halves) so the first projection
            # matmuls can start after half the chunk has landed
            ts = []
            for half in range(2):
                t = sb.tile([128, 3, QB], BF16, tag=f"embT_{n}_{half}")
                nc.sync.dma_start(
                    out=t[:],
                    in_=embT_h[
                        half * 384 : (half + 1) * 384, n * QB : (n + 1) * QB
                    ].rearrange("(c p) w -> p c w", p=128),
                )
                ts.append(t)
            embT_sb[n] = ts

        with (
            tc.tile_pool(name="psA", bufs=1, space="PSUM") as psA,
            tc.tile_pool(name="psT", bufs=1, space="PSUM") as psT,
            tc.tile_pool(name="psO", bufs=2, space="PSUM") as psO,
            tc.tile_pool(name="psS", bufs=2, space="PSUM") as psS,
            tc.tile_pool(name="ptp", bufs=16) as ptp,
            tc.tile_pool(name="rcp", bufs=4) as rcp,
        ):
            oacc_tiles = {}
            pt_tiles = {}

            def proj_qk_chunk(n):
                qs = slice(n * QB, (n + 1) * QB)
                ps = psA.tile([128, QB], F32, tag="proj")
                for c in range(6):
                    nc.tensor.matmul(
                        ps[:, :],
                        lhsT=wts_all[:, c, 0:128],
                        rhs=embT_sb[n][c // 3][:, c % 3, :],
                        start=(c == 0),
                        stop=(c == 5),
                    )
                # evacuate + fold to fp8 DoubleRow layout (cross-base copies).
                # K folds first: later chunks feed the (0/1, j) score pairs
                # through kf before qf is needed. Chunk 0's K folds ride the
                # still-idle ACT engine so the first exp starts sooner.
                if n == 0:
                    nc.scalar.copy(kf_sb[:, 0, qs], ps[64:96, :])
                    nc.scalar.copy(kf_sb[:, 1, qs], ps[96:128, :])
                else:
                    nc.vector.tensor_copy(out=kf_sb[:, 0, qs], in_=ps[64:96, :])
                    nc.vector.tensor_copy(out=kf_sb[:, 1, qs], in_=ps[96:128, :])
                nc.vector.tensor_scalar_add(
                    qf_sb[:, 0, qs], ps[0:32, :], bias_sb[0:32, 0:1]
                )
                nc.vector.tensor_scalar_add(
                    qf_sb[:, 1, qs], ps[32:64, :], bias_sb[32:64, 0:1]
                )

            def proj_v_chunk(n):
                qs = slice(n * QB, (n + 1) * QB)
                ps = psA.tile([128, QB], F32, tag="proj")
                for c in range(6):
                    nc.tensor.matmul(
                        ps[0:64, :],
                        lhsT=wts_all[:, c, 128:192],
                        rhs=embT_sb[n][c // 3][:, c % 3, :],
                        start=(c == 0),
                        stop=(c == 5),
                    )
                nc.vector.tensor_scalar_add(
                    vt_sb[0:64, qs], ps[0:64, :], bias_sb[0:64, 1:2]
                )
                # V^T chunk -> 4 V' tiles (128, 64) via PE transpose
                vtp = psT.tile([128, 256], BF16, tag="vtp")
                for jj in range(4):
                    j = 4 * n + jj
                    nc.tensor.transpose(
                        vtp[:, jj * 64 : (jj + 1) * 64],
                        vt_sb[0:64, j * 128 : (j + 1) * 128],
                        ident_bf[0:64, 0:64],
                    )
                nc.vector.tensor_copy(
                    out=vv_sb[:, 4 * n : 4 * n + 4, 0:D],
                    in_=vtp[:].rearrange("p (j d) -> p j d", j=4),
                )

            def attn_pair(n, j):
                qs = slice(n * QB, (n + 1) * QB)
                sc = psS.tile([128, 1024], F32, tag="sc")
                # S^T tiles for k-tiles 2j and 2j+1, fp8 DoubleRow (0.5 cyc/row)
                for h in range(2):
                    kt = 2 * j + h
                    nc.tensor.matmul(
                        sc[:, h * 512 : (h + 1) * 512],
                        lhsT=kf_sb[:, :, kt * 128 : (kt + 1) * 128],
                        rhs=qf_sb[:, :, qs],
                        start=True,
                        stop=True,
                        perf_mode=DR,
                    )
                pt = ptp.tile([128, 1024], BF16, tag="pt")
                nc.scalar.activation(pt[:], sc[:], AF.Exp, scale=SCALE)
                pt_tiles[(n, j)] = pt

            def av_block(n):
                # flipped AV: P^T 128x128 slices stationary, V' 65-col moving.
                # sub is the OUTER loop: PSUM accumulation groups sharing one
                # bank must not interleave (hardware corrupts them otherwise).
                oacc = psO.tile([128, 4, D + 1], F32, tag="oacc", name=f"oacc{rep}_{n}")
                oacc_tiles[n] = oacc
                for sub in range(4):
                    for j in range(NKP):
                        pt = pt_tiles[(n, j)]
                        for h in range(2):
                            kt = 2 * j + h
                            nc.tensor.matmul(
                                oacc[:, sub, :],
                                lhsT=pt[
                                    :, h * 512 + sub * 128 : h * 512 + (sub + 1) * 128
                                ],
                                rhs=vv_sb[:, kt, :],
                                start=(j == 0 and h == 0),
                                stop=(j == NKP - 1 and h == 1),
                                skip_group_check=True,
                            )

            def out_block(n):
                # per-sub divide + DMA: recip(sub) runs as soon as sub's
                # accumulation group stops, overlapping the later subs' AV
                # chains; per-sub DMAs shorten the final-block tail.
                oacc = oacc_tiles[n]
                for sub in range(4):
                    t = n * 4 + sub
                    rc = rcp.tile([128, 1], F32, tag="rc")
                    nc.vector.reciprocal(rc[:], oacc[:, sub, D : D + 1])
                    nc.vector.tensor_scalar_mul(
                        out_sb[:, t, :], oacc[:, sub, 0:D], rc[:, 0:1]
                    )
                    nc.sync.dma_start(
                        out=out_h[t * 128 : (t + 1) * 128, :],
                        in_=out_sb[:, t, :],
                    )

            # ---- software-pipelined emission, paced by chunk DMA arrival:
            # all four QK projections (+fp8 folds) first with score pairs
            # emitted as their kf/qf chunks complete, V projections deferred
            # (only needed by the AV stage), AV blocks at pt-pool turnover
            # points. reps > 1 repeats the whole computation for benchmarking.
            for rep in range(reps):
                dma_embT_chunk(0)
                if rep == 0:
                    nc.sync.dma_start(out=bias_sb[:], in_=bias_h[:])
                for n in range(1, NQB):
                    dma_embT_chunk(n)
                if rep == 0 and do_attn:
                    # PE warm-up during the DMA lead-in: dummy matmuls ramp the
                    # HAM clock gate (1.2 -> 2.4 GHz) before the first real
                    # projection matmul.
                    wmm = psS.tile([128, 1024], F32, tag="sc", name="warmmm")
                    for i in range(12):
                        nc.tensor.matmul(
                            wmm[:, (i % 2) * 512 : (i % 2) * 512 + 128],
                            lhsT=ident_bf[:, 0:128],
                            rhs=ident_bf[:, 0:128],
                            start=True,
                            stop=True,
                        )
                if do_proj:
                    proj_qk_chunk(0)
                if do_attn:
                    attn_pair(0, 0)
                    attn_pair(0, 1)
                if do_proj:
                    proj_qk_chunk(1)
                if do_attn:
                    attn_pair(0, 2)
                    attn_pair(0, 3)
                    for j in range(4):
                        attn_pair(1, j)
                if do_proj:
                    proj_qk_chunk(2)
                if do_attn:
                    attn_pair(0, 4)
                    attn_pair(0, 5)
                    attn_pair(1, 4)
                    attn_pair(1, 5)
                if do_proj:
                    proj_qk_chunk(3)
                if do_attn:
                    attn_pair(0, 6)
                    attn_pair(0, 7)
                    attn_pair(1, 6)
                    attn_pair(1, 7)
                if do_proj:
                    for n in range(NQB):
                        proj_v_chunk(n)
                if do_attn:
                    av_block(0)
                    if do_out:
                        out_block(0)
                    for j in range(NKP):
                        attn_pair(2, j)
                    av_block(1)
                    if do_out:
                        out_block(1)
                    for j in range(NKP):
                        attn_pair(3, j)
                    av_block(2)
                    if do_out:
                        out_block(2)
                    av_block(3)
                    if do_out:
                        out_block(3)
            if not do_out:
                nc.gpsimd.memset(out_sb[:, 0:1, :], 0.0)
                nc.sync.dma_start(
                    out=out_h[:].rearrange("(t p) i -> p t i", p=128),
                    in_=out_sb[:],
                )

    split_multi_waits(nc)
    return nc


_NC_CACHE = None


def _get_nc():
    global _NC_CACHE
    if _NC_CACHE is None:
        _NC_CACHE = build_nc()
    return _NC_CACHE


def make_in_maps(emb_input, Wq, bq, Wk, bk, Wv, bv):
    bf16 = ml_dtypes.bfloat16
    WqT = np.ascontiguousarray(Wq.T).astype(bf16)  # (768, 64)
    WkT = np.ascontiguousarray(Wk.T).astype(bf16)
    WvT = np.ascontiguousarray(Wv.T).astype(bf16)
    wts = np.ascontiguousarray(
        np.concatenate([WqT, WkT, WvT], axis=1)
    )  # (768, 192)
    biases = np.zeros((128, 2), np.float32)
    biases[0:64, 0] = bq
    biases[0:64, 1] = bv
    in_maps = []
    for i in range(NCORES):
        embT = np.ascontiguousarray(emb_input[i].T).astype(bf16)  # (768, 2048)
        in_maps.append({"embT": embT, "wts": wts, "biases": biases})
    return in_maps


def run(emb_input, Wq, bq, Wk, bk, Wv, bv, trace=False):
    nc = _get_nc()
    in_maps = make_in_maps(emb_input, Wq, bq, Wk, bk, Wv, bv)
    res = run_bass_kernel_spmd(nc, in_maps, core_ids=list(range(NCORES)), trace=trace)
    out = np.stack([res.results[i]["out"] for i in range(NCORES)], axis=0)
    return out.astype(np.float32), res


def kernel(emb_input, Wq, bq, Wk, bk, Wv, bv):
    out, _ = run(emb_input, Wq, bq, Wk, bk, Wv, bv, trace=False)
    return out


# revision 20
# speedup vs baseline: 1.2219x; 1.1362x over previous
"""Trainium2 Bass kernel for a single attention head.

Reference math (per batch b):
    q = emb @ Wq.T + bq ; k = emb @ Wk.T + bk ; v = emb @ Wv.T + bv
    attn = softmax((q @ k.T) / sqrt(768), axis=-1)
    out  = attn @ v

Sharding: pure data-parallel over batch. B=8 batches onto 8 NeuronCores,
one batch per core, no collectives.

Device-side layout strategy (per core):
  - emb arrives pre-transposed from the host as embT (768, 2048) bf16, one
    DMA per 512-column chunk ([128, 6, 512] tiles), so the E=768 contraction
    dim sits on SBUF partitions and HWDGE dispatch stays off the critical
    path.
  - Q and K projections are FUSED: one stationary [WqT|WkT] (768, 128) block
    computes Q^T (PSUM rows 0:64) and K^T (rows 64:128) in a single moving
    pass over each embT chunk.
  - bk is dropped: softmax over k is invariant to it.
  - Q^T/K^T are evacuated to fp8e4m3 in the DoubleRow-folded layout
    [32, 2, S] (plane j holds head dims j*32:(j+1)*32) via cross-
    partition-base DVE/Pool copies; the score matmuls then run in fp8
    DoubleRow perf mode at 0.5 cycles/row (2x bf16). Adds ~1% rel err
    (measured 1.34e-2 total vs the 2e-2 gate).
  - scores are computed transposed, S^T[k, q]; exp(S^T) (ACT, scale folded
    in) feeds the AV stage as bf16. No max-subtraction: scores*scale has
    std ~0.3, exp is safe in f32.
  - AV runs FLIPPED: P^T 128x128 slices are the stationary operand and
    V' (64 cols of V + an all-ones column for the softmax denominator Z)
    is the 65-column moving operand, accumulated over the 16 k-tiles.
    The output lands q-on-partitions, so no output transposes are needed;
    the final divide folds Z out directly from PSUM.
"""

import sys

import numpy as np

try:
    import concourse.bass as bass  # noqa: F401
except ImportError:  # pragma: no cover
    sys.path.insert(0, "/opt/trn_rl_repo")

from contextlib import ExitStack

import ml_dtypes

import concourse.bass as bass
import concourse.tile as tile
from concourse import mybir
from concourse.bass_utils import run_bass_kernel_spmd
from concourse.masks import make_identity

S = 2048  # sequence length
E = 768  # embedding dim
D = 64  # inner (head) dim
NCORES = 8
SCALE = float(1.0 / np.sqrt(np.float32(768.0)))

F32 = mybir.dt.float32
BF16 = mybir.dt.bfloat16
F8 = mybir.dt.float8e4
AF = mybir.ActivationFunctionType
DR = mybir.MatmulPerfMode.DoubleRow

QB = 512  # q block (one PSUM bank of f32)
NQB = S // QB  # 4 q blocks
NKT = S // 128  # 16 k tiles of 128
NKP = NKT // 2  # 8 k tile pairs


def split_multi_waits(nc: bass.Bass) -> int:
    """This toolchain's walrus encodes at most ONE semaphore wait per
    instruction ("Too many sync wait commands" otherwise). Tile freely emits
    multi-wait instructions, so hoist all but the last wait onto preceding
    same-engine NoOps — sequencer waits gate dispatch, so semantics are
    identical."""
    nsplit = 0
    for f in nc.m.functions:
        for bb in f.blocks:
            out = []
            changed = False
            for inst in bb.instructions:
                si = getattr(inst, "sync_info", None)
                if si is not None and len(si.on_wait) > 1:
                    waits = list(si.on_wait)
                    for w in waits[:-1]:
                        out.append(
                            mybir.InstNoOp(
                                name=nc.get_next_instruction_name(),
                                engine=inst.engine,
                                bass_nofuse=True,
                                sync_info=mybir.SyncInfo(on_wait=[w], on_update=[]),
                            )
                        )
                    inst.sync_info = mybir.SyncInfo(
                        on_wait=[waits[-1]], on_update=list(si.on_update)
                    )
                    changed = True
                    nsplit += 1
                out.append(inst)
            if changed:
                bb.instructions = out
    return nsplit


def build_nc(variant: str = "full", reps: int = 1) -> bass.Bass:
    do_proj = variant in ("full", "proj", "projattn")
    do_attn = variant in ("full", "projattn")
    do_out = variant == "full"
    nc = bass.Bass()

    embT_h = nc.declare_dram_parameter("embT", [E, S], BF16, isOutput=False)
    # host pre-arranges weights partition-major: [128, 6*192] so the DMA is
    # one contiguous 2304B run per partition (descriptor-count bound otherwise)
    wts_h = nc.declare_dram_parameter("wts", [128, 1152], BF16, isOutput=False)
    bias_h = nc.declare_dram_parameter("biases", [128, 2], F32, isOutput=False)
    out_h = nc.declare_dram_parameter("out", [S, D], F32, isOutput=True)

    with tile.TileContext(nc) as tc, ExitStack() as ctx:
        const = ctx.enter_context(tc.tile_pool(name="const", bufs=1))
        sb = ctx.enter_context(tc.tile_pool(name="sb", bufs=1))

        # ---- constants / small inputs ----
        # weights first: one DMA, (128, 1152) -> (128, 6, 192): [WqT|WkT|WvT]
        wts_all = const.tile([128, 6, 192], BF16, tag="wts")
        nc.sync.dma_start(
            out=wts_all[:], in_=wts_h[:].rearrange("p (c w) -> p c w", c=6)
        )
        bias_sb = const.tile([128, 2], F32, tag="bias")
        ident_bf = const.tile([128, 128], BF16, tag="idbf")
        make_identity(nc, ident_bf[:])

        # warm the ACT exp table set while DMAs run
        warm = const.tile([128, 8], F32, tag="warm")
        nc.gpsimd.memset(warm[:], 0.0)
        nc.scalar.activation(warm[:], warm[:], AF.Exp)

        # ---- persistent SBUF ----
        # Q^T / K^T in fp8 DoubleRow-folded layout: [32, 2, S], plane j holds
        # head dims j*32:(j+1)*32.
        qf_sb = sb.tile([32, 2, S], F8, tag="qf")
        kf_sb = sb.tile([32, 2, S], F8, tag="kf")
        vt_sb = sb.tile([64, S], BF16, tag="vt")
        # V' tiles: (k-tile, 65) with col 64 == 1.0 (softmax denominator)
        vv_sb = sb.tile([128, NKT, D + 1], BF16, tag="vv")
        nc.gpsimd.memset(vv_sb[:, :, D : D + 1], 1.0)
        out_sb = sb.tile([128, NKT, D], F32, tag="outsb")

        embT_sb = [None] * NQB

        def dma_embT_chunk(n, engine=None):
            # two DMAs per 512-col chunk (e-halves) so the first projection
            # matmuls can start after half the chunk has landed. Chunk 0 is
            # dispatched from the idle ACT queue, overlapping SP's wts
            # dispatch (the HWDGE dispatch slots are 625ns each, serial).
            eng = engine or nc.sync
            ts = []
            for half in range(2):
                t = sb.tile([128, 3, QB], BF16, tag=f"embT_{n}_{half}")
                eng.dma_start(
                    out=t[:],
                    in_=embT_h[
                        half * 384 : (half + 1) * 384, n * QB : (n + 1) * QB
                    ].rearrange("(c p) w -> p c w", p=128),
                )
                ts.append(t)
            embT_sb[n] = ts

        with (
            tc.tile_pool(name="psA", bufs=1, space="PSUM") as psA,
            tc.tile_pool(name="psT", bufs=1, space="PSUM") as psT,
            tc.tile_pool(name="psO", bufs=2, space="PSUM") as psO,
            tc.tile_pool(name="psS", bufs=2, space="PSUM") as psS,
            tc.tile_pool(name="ptp", bufs=24) as ptp,
            tc.tile_pool(name="rcp", bufs=4) as rcp,
        ):
            oacc_tiles = {}
            pt_tiles = {}

            def proj_qk_chunk(n):
                qs = slice(n * QB, (n + 1) * QB)
                ps = psA.tile([128, QB], F32, tag="proj")
                for c in range(6):
                    nc.tensor.matmul(
                        ps[:, :],
                        lhsT=wts_all[:, c, 0:128],
                        rhs=embT_sb[n][c // 3][:, c % 3, :],
                        start=(c == 0),
                        stop=(c == 5),
                    )
                # evacuate + fold to fp8 DoubleRow layout (cross-base copies).
                # K folds first: later chunks feed the (0/1, j) score pairs
                # through kf before qf is needed. Chunk 0's K folds ride the
                # still-idle ACT engine so the first exp starts sooner.
                if n == 0:
                    nc.scalar.copy(kf_sb[:, 0, qs], ps[64:96, :])
                    nc.scalar.copy(kf_sb[:, 1, qs], ps[96:128, :])
                else:
                    nc.vector.tensor_copy(out=kf_sb[:, 0, qs], in_=ps[64:96, :])
                    nc.vector.tensor_copy(out=kf_sb[:, 1, qs], in_=ps[96:128, :])
                nc.vector.tensor_scalar_add(
                    qf_sb[:, 0, qs], ps[0:32, :], bias_sb[0:32, 0:1]
                )
                nc.vector.tensor_scalar_add(
                    qf_sb[:, 1, qs], ps[32:64, :], bias_sb[32:64, 0:1]
                )

            def proj_v_chunk(n):
                qs = slice(n * QB, (n + 1) * QB)
                ps = psA.tile([128, QB], F32, tag="proj")
                for c in range(6):
                    nc.tensor.matmul(
                        ps[0:64, :],
                        lhsT=wts_all[:, c, 128:192],
                        rhs=embT_sb[n][c // 3][:, c % 3, :],
                        start=(c == 0),
                        stop=(c == 5),
                    )
                nc.vector.tensor_scalar_add(
                    vt_sb[0:64, qs], ps[0:64, :], bias_sb[0:64, 1:2]
                )
                # V^T chunk -> 4 V' tiles (128, 64) via PE transpose
                vtp = psT.tile([128, 256], BF16, tag="vtp")
                for jj in range(4):
                    j = 4 * n + jj
                    nc.tensor.transpose(
                        vtp[:, jj * 64 : (jj + 1) * 64],
                        vt_sb[0:64, j * 128 : (j + 1) * 128],
                        ident_bf[0:64, 0:64],
                    )
                nc.vector.tensor_copy(
                    out=vv_sb[:, 4 * n : 4 * n + 4, 0:D],
                    in_=vtp[:].rearrange("p (j d) -> p j d", j=4),
                )

            def attn_pair(n, j):
                qs = slice(n * QB, (n + 1) * QB)
                sc = psS.tile([128, 1024], F32, tag="sc")
                # S^T tiles for k-tiles 2j and 2j+1, fp8 DoubleRow (0.5 cyc/row)
                for h in range(2):
                    kt = 2 * j + h
                    nc.tensor.matmul(
                        sc[:, h * 512 : (h + 1) * 512],
                        lhsT=kf_sb[:, :, kt * 128 : (kt + 1) * 128],
                        rhs=qf_sb[:, :, qs],
                        start=True,
                        stop=True,
                        perf_mode=DR,
                    )
                pt = ptp.tile([128, 1024], BF16, tag="pt")
                nc.scalar.activation(pt[:], sc[:], AF.Exp, scale=SCALE)
                pt_tiles[(n, j)] = pt

            def av_block(n):
                # flipped AV: P^T 128x128 slices stationary, V' 65-col moving.
                # sub is the OUTER loop: PSUM accumulation groups sharing one
                # bank must not interleave (hardware corrupts them otherwise).
                oacc = psO.tile([128, 4, D + 1], F32, tag="oacc", name=f"oacc{rep}_{n}")
                oacc_tiles[n] = oacc
                for sub in range(4):
                    for j in range(NKP):
                        pt = pt_tiles[(n, j)]
                        for h in range(2):
                            kt = 2 * j + h
                            nc.tensor.matmul(
                                oacc[:, sub, :],
                                lhsT=pt[
                                    :, h * 512 + sub * 128 : h * 512 + (sub + 1) * 128
                                ],
                                rhs=vv_sb[:, kt, :],
                                start=(j == 0 and h == 0),
                                stop=(j == NKP - 1 and h == 1),
                                skip_group_check=True,
                            )

            def out_block(n):
                # per-sub divide + DMA: recip(sub) runs as soon as sub's
                # accumulation group stops, overlapping the later subs' AV
                # chains; per-sub DMAs shorten the final-block tail.
                oacc = oacc_tiles[n]
                for sub in range(4):
                    t = n * 4 + sub
                    rc = rcp.tile([128, 1], F32, tag="rc")
                    nc.vector.reciprocal(rc[:], oacc[:, sub, D : D + 1])
                    nc.vector.tensor_scalar_mul(
                        out_sb[:, t, :], oacc[:, sub, 0:D], rc[:, 0:1]
                    )
                    nc.sync.dma_start(
                        out=out_h[t * 128 : (t + 1) * 128, :],
                        in_=out_sb[:, t, :],
                    )

            # ---- software-pipelined emission, paced by chunk DMA arrival:
            # all four QK projections (+fp8 folds) first with score pairs
            # emitted as their kf/qf chunks complete, V projections deferred
            # (only needed by the AV stage), AV blocks at pt-pool turnover
            # points. reps > 1 repeats the whole computation for benchmarking.
            for rep in range(reps):
                dma_embT_chunk(0, engine=nc.scalar if rep == 0 else None)
                if rep == 0:
                    nc.sync.dma_start(out=bias_sb[:], in_=bias_h[:])
                for n in range(1, NQB):
                    dma_embT_chunk(n)
                if rep == 0 and do_attn:
                    # PE warm-up during the DMA lead-in: dummy matmuls ramp the
                    # HAM clock gate (1.2 -> 2.4 GHz) and keep the PE busy
                    # until the first embT half lands.
                    wmm = psS.tile([128, 1024], F32, tag="sc", name="warmmm")
                    for i in range(22):
                        nc.tensor.matmul(
                            wmm[:, (i % 2) * 512 : (i % 2) * 512 + 128],
                            lhsT=ident_bf[:, 0:128],
                            rhs=ident_bf[:, 0:128],
                            start=True,
                            stop=True,
                        )
                if do_proj:
                    proj_qk_chunk(0)
                if do_attn:
                    attn_pair(0, 0)
                    attn_pair(0, 1)
                if do_proj:
                    proj_qk_chunk(1)
                if do_attn:
                    attn_pair(0, 2)
                    attn_pair(0, 3)
                    for j in range(4):
                        attn_pair(1, j)
                if do_proj:
                    proj_qk_chunk(2)
                if do_attn:
                    attn_pair(0, 4)
                    attn_pair(0, 5)
                    attn_pair(1, 4)
                    attn_pair(1, 5)
                if do_proj:
                    proj_qk_chunk(3)
                if do_attn:
                    attn_pair(0, 6)
                    attn_pair(0, 7)
                    attn_pair(1, 6)
                    attn_pair(1, 7)
                if do_proj:
                    for n in range(NQB):
                        proj_v_chunk(n)
                if do_attn:
                    for j in range(NKP):
                        attn_pair(2, j)
                    av_block(0)
                    if do_out:
                        out_block(0)
                    for j in range(4):
                        attn_pair(3, j)
                    av_block(1)
                    if do_out:
                        out_block(1)
                    for j in range(4, NKP):
                        attn_pair(3, j)
                    av_block(2)
                    if do_out:
                        out_block(2)
                    av_block(3)
                    if do_out:
                        out_block(3)
            if not do_out:
                nc.gpsimd.memset(out_sb[:, 0:1, :], 0.0)
                nc.sync.dma_start(
                    out=out_h[:].rearrange("(t p) i -> p t i", p=128),
                    in_=out_sb[:],
                )

    split_multi_waits(nc)
    return nc


_NC_CACHE = None


def _get_nc():
    global _NC_CACHE
    if _NC_CACHE is None:
        _NC_CACHE = build_nc()
    return _NC_CACHE


def make_in_maps(emb_input, Wq, bq, Wk, bk, Wv, bv):
    bf16 = ml_dtypes.bfloat16
    WqT = np.ascontiguousarray(Wq.T).astype(bf16)  # (768, 64)
    WkT = np.ascontiguousarray(Wk.T).astype(bf16)
    WvT = np.ascontiguousarray(Wv.T).astype(bf16)
    wts = np.concatenate([WqT, WkT, WvT], axis=1)  # (768, 192)
    # partition-major: (6, 128, 192) -> (128, 6*192) contiguous per partition
    wts = np.ascontiguousarray(
        wts.reshape(6, 128, 192).transpose(1, 0, 2).reshape(128, 1152)
    )
    biases = np.zeros((128, 2), np.float32)
    biases[0:64, 0] = bq
    biases[0:64, 1] = bv
    in_maps = []
    for i in range(NCORES):
        embT = np.ascontiguousarray(emb_input[i].T).astype(bf16)  # (768, 2048)
        in_maps.append({"embT": embT, "wts": wts, "biases": biases})
    return in_maps


def run(emb_input, Wq, bq, Wk, bk, Wv, bv, trace=False):
    nc = _get_nc()
    in_maps = make_in_maps(emb_input, Wq, bq, Wk, bk, Wv, bv)
    res = run_bass_kernel_spmd(nc, in_maps, core_ids=list(range(NCORES)), trace=trace)
    out = np.stack([res.results[i]["out"] for i in range(NCORES)], axis=0)
    return out.astype(np.float32), res


def kernel(emb_input, Wq, bq, Wk, bk, Wv, bv):
    out, _ = run(emb_input, Wq, bq, Wk, bk, Wv, bv, trace=False)
    return out


# revision 27
# speedup vs baseline: 1.3183x; 1.0789x over previous
"""Trainium2 Bass kernel for a single attention head.

Reference math (per batch b):
    q = emb @ Wq.T + bq ; k = emb @ Wk.T + bk ; v = emb @ Wv.T + bv
    attn = softmax((q @ k.T) / sqrt(768), axis=-1)
    out  = attn @ v

Sharding: pure data-parallel over batch. B=8 batches onto 8 NeuronCores,
one batch per core, no collectives.

Device-side layout strategy (per core):
  - emb arrives pre-transposed from the host as embT (768, 2048) bf16, one
    DMA per 512-column chunk ([128, 6, 512] tiles), so the E=768 contraction
    dim sits on SBUF partitions and HWDGE dispatch stays off the critical
    path.
  - Q and K projections are FUSED: one stationary [WqT|WkT] (768, 128) block
    computes Q^T (PSUM rows 0:64) and K^T (rows 64:128) in a single moving
    pass over each embT chunk.
  - bk is dropped: softmax over k is invariant to it.
  - Q^T/K^T are evacuated to fp8e4m3 in the DoubleRow-folded layout
    [32, 2, S] (plane j holds head dims j*32:(j+1)*32) via cross-
    partition-base DVE/Pool copies; the score matmuls then run in fp8
    DoubleRow perf mode at 0.5 cycles/row (2x bf16). Adds ~1% rel err
    (measured 1.34e-2 total vs the 2e-2 gate).
  - scores are computed transposed, S^T[k, q]; exp(S^T) (ACT, scale folded
    in) feeds the AV stage as bf16. No max-subtraction: scores*scale has
    std ~0.3, exp is safe in f32.
  - AV runs FLIPPED: P^T 128x128 slices are the stationary operand and
    V' (64 cols of V + an all-ones column for the softmax denominator Z)
    is the 65-column moving operand, accumulated over the 16 k-tiles.
    The output lands q-on-partitions, so no output transposes are needed;
    the final divide folds Z out directly from PSUM.
"""

import sys

import numpy as np

try:
    import concourse.bass as bass  # noqa: F401
except ImportError:  # pragma: no cover
    sys.path.insert(0, "/opt/trn_rl_repo")

from contextlib import ExitStack

import ml_dtypes

import concourse.bass as bass
import concourse.tile as tile
from concourse import mybir
from concourse.bass_utils import run_bass_kernel_spmd
from concourse.masks import make_identity

S = 2048  # sequence length
E = 768  # embedding dim
D = 64  # inner (head) dim
NCORES = 8
SCALE = float(1.0 / np.sqrt(np.float32(768.0)))

F32 = mybir.dt.float32
BF16 = mybir.dt.bfloat16
F8 = mybir.dt.float8e4
AF = mybir.ActivationFunctionType
DR = mybir.MatmulPerfMode.DoubleRow

QB = 512  # q block (one PSUM bank of f32)
NQB = S // QB  # 4 q blocks
NKT = S // 128  # 16 k tiles of 128
NKP = NKT // 2  # 8 k tile pairs


def split_multi_waits(nc: bass.Bass) -> int:
    """This toolchain's walrus encodes at most ONE semaphore wait per
    instruction ("Too many sync wait commands" otherwise). Tile freely emits
    multi-wait instructions, so hoist all but the last wait onto preceding
    same-engine NoOps — sequencer waits gate dispatch, so semantics are
    identical."""
    nsplit = 0
    for f in nc.m.functions:
        for bb in f.blocks:
            out = []
            changed = False
            for inst in bb.instructions:
                si = getattr(inst, "sync_info", None)
                if si is not None and len(si.on_wait) > 1:
                    waits = list(si.on_wait)
                    for w in waits[:-1]:
                        out.append(
                            mybir.InstNoOp(
                                name=nc.get_next_instruction_name(),
                                engine=inst.engine,
                                bass_nofuse=True,
                                sync_info=mybir.SyncInfo(on_wait=[w], on_update=[]),
                            )
                        )
                    inst.sync_info = mybir.SyncInfo(
                        on_wait=[waits[-1]], on_update=list(si.on_update)
                    )
                    changed = True
                    nsplit += 1
                out.append(inst)
            if changed:
                bb.instructions = out
    return nsplit


def build_nc(variant: str = "full", reps: int = 1) -> bass.Bass:
    do_proj = variant in ("full", "proj", "projattn")
    do_attn = variant in ("full", "projattn")
    do_out = variant == "full"
    nc = bass.Bass()

    embT_h = nc.declare_dram_parameter("embT", [E, S], BF16, isOutput=False)
    # host pre-arranges weights partition-major (contiguous per partition);
    # QK and V blocks are separate DMAs so the QK projections start sooner
    wqk_h = nc.declare_dram_parameter("wqk", [128, 768], BF16, isOutput=False)
    wv_h = nc.declare_dram_parameter("wv", [128, 384], BF16, isOutput=False)
    bias_h = nc.declare_dram_parameter("biases", [128, 2], F32, isOutput=False)
    out_h = nc.declare_dram_parameter("out", [S, D], F32, isOutput=True)

    with tile.TileContext(nc) as tc, ExitStack() as ctx:
        const = ctx.enter_context(tc.tile_pool(name="const", bufs=1))
        sb = ctx.enter_context(tc.tile_pool(name="sb", bufs=1))

        # ---- constants / small inputs ----
        # QK weights first: (128, 768) -> (128, 6, 128) [WqT|WkT]
        wqk_sb = const.tile([128, 6, 128], BF16, tag="wqk")
        nc.sync.dma_start(
            out=wqk_sb[:], in_=wqk_h[:].rearrange("p (c w) -> p c w", c=6)
        )
        wv_sb = const.tile([128, 6, 64], BF16, tag="wv")
        bias_sb = const.tile([128, 2], F32, tag="bias")
        ident_bf = const.tile([128, 128], BF16, tag="idbf")
        make_identity(nc, ident_bf[:])

        # warm the ACT exp table set while DMAs run
        warm = const.tile([128, 8], F32, tag="warm")
        nc.gpsimd.memset(warm[:], 0.0)
        nc.scalar.activation(warm[:], warm[:], AF.Exp)

        # ---- persistent SBUF ----
        # Q^T / K^T in fp8 zero-plane DoubleRow layout: [64, 2, S], plane 1
        # is all zeros (memset once). The DoubleRow matmul charges 0.5
        # cycles/row on the output columns, so the dead plane costs nothing.
        qf_sb = sb.tile([64, 2, S], F8, tag="qf")
        kf_sb = sb.tile([64, 2, S], F8, tag="kf")
        nc.gpsimd.memset(qf_sb[:, 1, :], 0.0)
        nc.gpsimd.memset(kf_sb[:, 1, :], 0.0)
        vt_sb = sb.tile([64, S], BF16, tag="vt")
        # V' tiles: (k-tile, 65) with col 64 == 1.0 (softmax denominator)
        vv_sb = sb.tile([128, NKT, D + 1], BF16, tag="vv")
        nc.gpsimd.memset(vv_sb[:, :, D : D + 1], 1.0)
        out_sb = sb.tile([128, NKT, D], F32, tag="outsb")

        embT_sb = [None] * NQB

        def dma_embT_chunk(n):
            # two DMAs per 512-col chunk (e-halves) so the first projection
            # matmuls can start after half the chunk has landed
            ts = []
            for half in range(2):
                t = sb.tile([128, 3, QB], BF16, tag=f"embT_{n}_{half}")
                nc.sync.dma_start(
                    out=t[:],
                    in_=embT_h[
                        half * 384 : (half + 1) * 384, n * QB : (n + 1) * QB
                    ].rearrange("(c p) w -> p c w", p=128),
                )
                ts.append(t)
            embT_sb[n] = ts

        with (
            tc.tile_pool(name="psA", bufs=1, space="PSUM") as psA,
            tc.tile_pool(name="psT", bufs=1, space="PSUM") as psT,
            tc.tile_pool(name="psO", bufs=2, space="PSUM") as psO,
            tc.tile_pool(name="psS", bufs=2, space="PSUM") as psS,
            tc.tile_pool(name="ptp", bufs=24) as ptp,
            tc.tile_pool(name="rcp", bufs=4) as rcp,
        ):
            oacc_tiles = {}
            pt_tiles = {}

            def proj_qk_chunk(n):
                qs = slice(n * QB, (n + 1) * QB)
                ps = psA.tile([128, QB], F32, tag="proj")
                for c in range(6):
                    nc.tensor.matmul(
                        ps[:, :],
                        lhsT=wqk_sb[:, c, :],
                        rhs=embT_sb[n][c // 3][:, c % 3, :],
                        start=(c == 0),
                        stop=(c == 5),
                    )
                # evacuate to fp8 plane 0. K first: later chunks feed the
                # (0/1, j) score pairs through kf before qf is needed.
                # Chunk 0's K evac rides the still-idle ACT engine.
                if n == 0:
                    nc.scalar.copy(kf_sb[:, 0, qs], ps[64:128, :])
                else:
                    nc.vector.tensor_copy(out=kf_sb[:, 0, qs], in_=ps[64:128, :])
                nc.vector.tensor_scalar_add(
                    qf_sb[:, 0, qs], ps[0:64, :], bias_sb[0:64, 0:1]
                )

            def proj_v_chunk(n):
                qs = slice(n * QB, (n + 1) * QB)
                ps = psA.tile([128, QB], F32, tag="proj")
                for c in range(6):
                    nc.tensor.matmul(
                        ps[0:64, :],
                        lhsT=wv_sb[:, c, :],
                        rhs=embT_sb[n][c // 3][:, c % 3, :],
                        start=(c == 0),
                        stop=(c == 5),
                    )
                nc.vector.tensor_scalar_add(
                    vt_sb[0:64, qs], ps[0:64, :], bias_sb[0:64, 1:2]
                )
                # V^T chunk -> 4 V' tiles (128, 64) via PE transpose
                vtp = psT.tile([128, 256], BF16, tag="vtp")
                for jj in range(4):
                    j = 4 * n + jj
                    nc.tensor.transpose(
                        vtp[:, jj * 64 : (jj + 1) * 64],
                        vt_sb[0:64, j * 128 : (j + 1) * 128],
                        ident_bf[0:64, 0:64],
                    )
                nc.vector.tensor_copy(
                    out=vv_sb[:, 4 * n : 4 * n + 4, 0:D],
                    in_=vtp[:].rearrange("p (j d) -> p j d", j=4),
                )

            def attn_pair(n, j):
                qs = slice(n * QB, (n + 1) * QB)
                sc = psS.tile([128, 1024], F32, tag="sc")
                # S^T tiles for k-tiles 2j and 2j+1, fp8 DoubleRow (0.5 cyc/row)
                for h in range(2):
                    kt = 2 * j + h
                    nc.tensor.matmul(
                        sc[:, h * 512 : (h + 1) * 512],
                        lhsT=kf_sb[:, :, kt * 128 : (kt + 1) * 128],
                        rhs=qf_sb[:, :, qs],
                        start=True,
                        stop=True,
                        perf_mode=DR,
                    )
                pt = ptp.tile([128, 1024], BF16, tag="pt")
                nc.scalar.activation(pt[:], sc[:], AF.Exp, scale=SCALE)
                pt_tiles[(n, j)] = pt

            def av_block(n):
                # flipped AV: P^T 128x128 slices stationary, V' 65-col moving.
                # sub is the OUTER loop: PSUM accumulation groups sharing one
                # bank must not interleave (hardware corrupts them otherwise).
                oacc = psO.tile([128, 4, D + 1], F32, tag="oacc", name=f"oacc{rep}_{n}")
                oacc_tiles[n] = oacc
                for sub in range(4):
                    for j in range(NKP):
                        pt = pt_tiles[(n, j)]
                        for h in range(2):
                            kt = 2 * j + h
                            nc.tensor.matmul(
                                oacc[:, sub, :],
                                lhsT=pt[
                                    :, h * 512 + sub * 128 : h * 512 + (sub + 1) * 128
                                ],
                                rhs=vv_sb[:, kt, :],
                                start=(j == 0 and h == 0),
                                stop=(j == NKP - 1 and h == 1),
                                skip_group_check=True,
                            )

            def out_block(n):
                # per-sub divide (recip(sub) runs as soon as sub's
                # accumulation group stops), one DMA per block
                oacc = oacc_tiles[n]
                for sub in range(4):
                    t = n * 4 + sub
                    rc = rcp.tile([128, 1], F32, tag="rc")
                    nc.vector.reciprocal(rc[:], oacc[:, sub, D : D + 1])
                    nc.vector.tensor_scalar_mul(
                        out_sb[:, t, :], oacc[:, sub, 0:D], rc[:, 0:1]
                    )
                qs = slice(n * QB, (n + 1) * QB)
                nc.sync.dma_start(
                    out=out_h[qs, :].rearrange("(t p) i -> p t i", p=128),
                    in_=out_sb[:, n * 4 : (n + 1) * 4, :],
                )

            # ---- software-pipelined emission, paced by chunk DMA arrival:
            # all four QK projections (+fp8 folds) first with score pairs
            # emitted as their kf/qf chunks complete, V projections deferred
            # (only needed by the AV stage), AV blocks at pt-pool turnover
            # points. reps > 1 repeats the whole computation for benchmarking.
            for rep in range(reps):
                dma_embT_chunk(0)
                if rep == 0:
                    nc.sync.dma_start(out=bias_sb[:], in_=bias_h[:])
                    nc.sync.dma_start(
                        out=wv_sb[:],
                        in_=wv_h[:].rearrange("p (c w) -> p c w", c=6),
                    )
                for n in range(1, NQB):
                    dma_embT_chunk(n)
                if rep == 0 and do_attn:
                    # PE warm-up during the DMA lead-in: dummy matmuls ramp the
                    # HAM clock gate (1.2 -> 2.4 GHz) and keep the PE busy
                    # until the first embT half lands.
                    wmm = psS.tile([128, 1024], F32, tag="sc", name="warmmm")
                    for i in range(22):
                        nc.tensor.matmul(
                            wmm[:, (i % 2) * 512 : (i % 2) * 512 + 128],
                            lhsT=ident_bf[:, 0:128],
                            rhs=ident_bf[:, 0:128],
                            start=True,
                            stop=True,
                        )
                if do_proj:
                    proj_qk_chunk(0)
                if do_attn:
                    attn_pair(0, 0)
                    attn_pair(0, 1)
                if do_proj:
                    proj_qk_chunk(1)
                if do_attn:
                    attn_pair(0, 2)
                    attn_pair(0, 3)
                    for j in range(4):
                        attn_pair(1, j)
                if do_proj:
                    proj_qk_chunk(2)
                if do_attn:
                    attn_pair(0, 4)
                    attn_pair(0, 5)
                    attn_pair(1, 4)
                    attn_pair(1, 5)
                if do_proj:
                    proj_qk_chunk(3)
                if do_attn:
                    attn_pair(0, 6)
                    attn_pair(0, 7)
                    attn_pair(1, 6)
                    attn_pair(1, 7)
                if do_proj:
                    for n in range(NQB):
                        proj_v_chunk(n)
                if do_attn:
                    for j in range(NKP):
                        attn_pair(2, j)
                    av_block(0)
                    if do_out:
                        out_block(0)
                    for j in range(4):
                        attn_pair(3, j)
                    av_block(1)
                    if do_out:
                        out_block(1)
                    for j in range(4, NKP):
                        attn_pair(3, j)
                    av_block(2)
                    if do_out:
                        out_block(2)
                    av_block(3)
                    if do_out:
                        out_block(3)
            if not do_out:
                nc.gpsimd.memset(out_sb[:, 0:1, :], 0.0)
                nc.sync.dma_start(
                    out=out_h[:].rearrange("(t p) i -> p t i", p=128),
                    in_=out_sb[:],
                )

    split_multi_waits(nc)
    return nc


_NC_CACHE = None


def _get_nc():
    global _NC_CACHE
    if _NC_CACHE is None:
        _NC_CACHE = build_nc()
    return _NC_CACHE


def make_in_maps(emb_input, Wq, bq, Wk, bk, Wv, bv):
    bf16 = ml_dtypes.bfloat16
    WqT = np.ascontiguousarray(Wq.T).astype(bf16)  # (768, 64)
    WkT = np.ascontiguousarray(Wk.T).astype(bf16)
    WvT = np.ascontiguousarray(Wv.T).astype(bf16)
    # partition-major: (6, 128, w) -> (128, 6*w) contiguous per partition
    wqk = np.concatenate([WqT, WkT], axis=1)  # (768, 128)
    wqk = np.ascontiguousarray(
        wqk.reshape(6, 128, 128).transpose(1, 0, 2).reshape(128, 768)
    )
    wv = np.ascontiguousarray(
        WvT.reshape(6, 128, 64).transpose(1, 0, 2).reshape(128, 384)
    )
    biases = np.zeros((128, 2), np.float32)
    biases[0:64, 0] = bq
    biases[0:64, 1] = bv
    in_maps = []
    for i in range(NCORES):
        embT = np.ascontiguousarray(emb_input[i].T).astype(bf16)  # (768, 2048)
        in_maps.append({"embT": embT, "wqk": wqk, "wv": wv, "biases": biases})
    return in_maps


def run(emb_input, Wq, bq, Wk, bk, Wv, bv, trace=False):
    nc = _get_nc()
    in_maps = make_in_maps(emb_input, Wq, bq, Wk, bk, Wv, bv)
    res = run_bass_kernel_spmd(nc, in_maps, core_ids=list(range(NCORES)), trace=trace)
    out = np.stack([res.results[i]["out"] for i in range(NCORES)], axis=0)
    return out.astype(np.float32), res


def kernel(emb_input, Wq, bq, Wk, bk, Wv, bv):
    out, _ = run(emb_input, Wq, bq, Wk, bk, Wv, bv, trace=False)
    return out


# revision 36
# speedup vs baseline: 1.3352x; 1.0128x over previous
"""Trainium2 Bass kernel for a single attention head.

Reference math (per batch b):
    q = emb @ Wq.T + bq ; k = emb @ Wk.T + bk ; v = emb @ Wv.T + bv
    attn = softmax((q @ k.T) / sqrt(768), axis=-1)
    out  = attn @ v

Sharding: pure data-parallel over batch. B=8 batches onto 8 NeuronCores,
one batch per core, no collectives.

Device-side layout strategy (per core):
  - emb arrives pre-transposed from the host as embT (768, 2048) bf16, one
    DMA per 512-column chunk ([128, 6, 512] tiles), so the E=768 contraction
    dim sits on SBUF partitions and HWDGE dispatch stays off the critical
    path.
  - Q and K projections are FUSED: one stationary [WqT|WkT] (768, 128) block
    computes Q^T (PSUM rows 0:64) and K^T (rows 64:128) in a single moving
    pass over each embT chunk.
  - bk is dropped: softmax over k is invariant to it.
  - Q^T/K^T are evacuated to fp8e4m3 in the DoubleRow-folded layout
    [32, 2, S] (plane j holds head dims j*32:(j+1)*32) via cross-
    partition-base DVE/Pool copies; the score matmuls then run in fp8
    DoubleRow perf mode at 0.5 cycles/row (2x bf16). Adds ~1% rel err
    (measured 1.34e-2 total vs the 2e-2 gate).
  - scores are computed transposed, S^T[k, q]; exp(S^T) (ACT, scale folded
    in) feeds the AV stage as bf16. No max-subtraction: scores*scale has
    std ~0.3, exp is safe in f32.
  - AV runs FLIPPED: P^T 128x128 slices are the stationary operand and
    V' (64 cols of V + an all-ones column for the softmax denominator Z)
    is the 65-column moving operand, accumulated over the 16 k-tiles.
    The output lands q-on-partitions, so no output transposes are needed;
    the final divide folds Z out directly from PSUM.
"""

import sys

import numpy as np

try:
    import concourse.bass as bass  # noqa: F401
except ImportError:  # pragma: no cover
    sys.path.insert(0, "/opt/trn_rl_repo")

from contextlib import ExitStack

import ml_dtypes

import concourse.bass as bass
import concourse.tile as tile
from concourse import mybir
from concourse.bass_utils import run_bass_kernel_spmd
from concourse.masks import make_identity

S = 2048  # sequence length
E = 768  # embedding dim
D = 64  # inner (head) dim
NCORES = 8
SCALE = float(1.0 / np.sqrt(np.float32(768.0)))

F32 = mybir.dt.float32
BF16 = mybir.dt.bfloat16
F8 = mybir.dt.float8e4
AF = mybir.ActivationFunctionType
DR = mybir.MatmulPerfMode.DoubleRow

QB = 512  # q block (one PSUM bank of f32)
NQB = S // QB  # 4 q blocks
NKT = S // 128  # 16 k tiles of 128
NKP = NKT // 2  # 8 k tile pairs


def split_multi_waits(nc: bass.Bass) -> int:
    """This toolchain's walrus encodes at most ONE semaphore wait per
    instruction ("Too many sync wait commands" otherwise). Tile freely emits
    multi-wait instructions, so hoist all but the last wait onto preceding
    same-engine NoOps — sequencer waits gate dispatch, so semantics are
    identical."""
    nsplit = 0
    for f in nc.m.functions:
        for bb in f.blocks:
            out = []
            changed = False
            for inst in bb.instructions:
                si = getattr(inst, "sync_info", None)
                if si is not None and len(si.on_wait) > 1:
                    waits = list(si.on_wait)
                    for w in waits[:-1]:
                        out.append(
                            mybir.InstNoOp(
                                name=nc.get_next_instruction_name(),
                                engine=inst.engine,
                                bass_nofuse=True,
                                sync_info=mybir.SyncInfo(on_wait=[w], on_update=[]),
                            )
                        )
                    inst.sync_info = mybir.SyncInfo(
                        on_wait=[waits[-1]], on_update=list(si.on_update)
                    )
                    changed = True
                    nsplit += 1
                out.append(inst)
            if changed:
                bb.instructions = out
    return nsplit


def build_nc(variant: str = "full", reps: int = 1) -> bass.Bass:
    do_proj = variant in ("full", "proj", "projattn")
    do_attn = variant in ("full", "projattn")
    do_out = variant == "full"
    nc = bass.Bass()

    embT_h = nc.declare_dram_parameter("embT", [E, S], BF16, isOutput=False)
    # host pre-arranges weights partition-major (contiguous per partition);
    # QK and V blocks are separate DMAs so the QK projections start sooner
    wqk_h = nc.declare_dram_parameter("wqk", [128, 768], BF16, isOutput=False)
    wv_h = nc.declare_dram_parameter("wv", [128, 384], BF16, isOutput=False)
    bias_h = nc.declare_dram_parameter("biases", [128, 2], F32, isOutput=False)
    out_h = nc.declare_dram_parameter("out", [S, D], F32, isOutput=True)

    with tile.TileContext(nc) as tc, ExitStack() as ctx:
        const = ctx.enter_context(tc.tile_pool(name="const", bufs=1))
        sb = ctx.enter_context(tc.tile_pool(name="sb", bufs=1))

        # ---- constants / small inputs ----
        # QK weights first: (128, 768) -> (128, 6, 128) [WqT|WkT]
        wqk_sb = const.tile([128, 6, 128], BF16, tag="wqk")
        nc.sync.dma_start(
            out=wqk_sb[:], in_=wqk_h[:].rearrange("p (c w) -> p c w", c=6)
        )
        wv_sb = const.tile([128, 6, 64], BF16, tag="wv")
        bias_sb = const.tile([128, 2], F32, tag="bias")
        ident_bf = const.tile([128, 128], BF16, tag="idbf")
        make_identity(nc, ident_bf[:])

        # warm the ACT exp table set while DMAs run
        warm = const.tile([128, 8], F32, tag="warm")
        nc.gpsimd.memset(warm[:], 0.0)
        nc.scalar.activation(warm[:], warm[:], AF.Exp)

        # ---- persistent SBUF ----
        # Q^T / K^T in fp8 zero-plane DoubleRow layout: [64, 2, S], plane 1
        # is all zeros (zeroed once on the idle ACT engine; memzero bitcasts
        # to uint32 so it is 512 elements, not 2048). The DoubleRow matmul
        # charges 0.5 cycles/row on the output columns, so the dead plane
        # costs nothing.
        qf_sb = sb.tile([64, 2, S], F8, tag="qf")
        kf_sb = sb.tile([64, 2, S], F8, tag="kf")
        nc.scalar.memzero(qf_sb[:, 1, :])
        nc.scalar.memzero(kf_sb[:, 1, :])
        vt_sb = sb.tile([64, S], BF16, tag="vt")
        # V' tiles: (k-tile, 65) with col 64 == 1.0 (softmax denominator)
        vv_sb = sb.tile([128, NKT, D + 1], BF16, tag="vv")
        nc.gpsimd.memset(vv_sb[:, :, D : D + 1], 1.0)
        out_sb = sb.tile([128, NKT, D], F32, tag="outsb")

        embT_sb = [None] * NQB

        def dma_embT_chunk(n):
            # three DMAs per 512-col chunk (e-thirds) so projection matmuls
            # start as soon as the first third lands
            ts = []
            for part in range(3):
                t = sb.tile([128, 2, QB], BF16, tag=f"embT_{n}_{part}")
                nc.sync.dma_start(
                    out=t[:],
                    in_=embT_h[
                        part * 256 : (part + 1) * 256, n * QB : (n + 1) * QB
                    ].rearrange("(c p) w -> p c w", p=128),
                )
                ts.append(t)
            embT_sb[n] = ts

        with (
            tc.tile_pool(name="psA", bufs=1, space="PSUM") as psA,
            tc.tile_pool(name="psT", bufs=1, space="PSUM") as psT,
            tc.tile_pool(name="psOA", bufs=1, space="PSUM") as psOA,
            tc.tile_pool(name="psOB", bufs=1, space="PSUM") as psOB,
            tc.tile_pool(name="psS", bufs=2, space="PSUM") as psS,
            tc.tile_pool(name="ptp", bufs=24) as ptp,
            tc.tile_pool(name="rcp", bufs=4) as rcp,
        ):
            oacc_tiles = {}
            pt_tiles = {}

            def proj_qk_chunk(n):
                qs = slice(n * QB, (n + 1) * QB)
                ps = psA.tile([128, QB], F32, tag="proj")
                for c in range(6):
                    nc.tensor.matmul(
                        ps[:, :],
                        lhsT=wqk_sb[:, c, :],
                        rhs=embT_sb[n][c // 2][:, c % 2, :],
                        start=(c == 0),
                        stop=(c == 5),
                    )
                # evacuate to fp8 plane 0. K first: later chunks feed the
                # (0/1, j) score pairs through kf before qf is needed.
                # Chunk 0's K evac rides the still-idle ACT engine.
                if n == 0:
                    nc.scalar.copy(kf_sb[:, 0, qs], ps[64:128, :])
                else:
                    nc.vector.tensor_copy(out=kf_sb[:, 0, qs], in_=ps[64:128, :])
                nc.vector.tensor_scalar_add(
                    qf_sb[:, 0, qs], ps[0:64, :], bias_sb[0:64, 0:1]
                )

            def proj_v_chunk(n):
                qs = slice(n * QB, (n + 1) * QB)
                ps = psA.tile([128, QB], F32, tag="proj")
                for c in range(6):
                    nc.tensor.matmul(
                        ps[0:64, :],
                        lhsT=wv_sb[:, c, :],
                        rhs=embT_sb[n][c // 2][:, c % 2, :],
                        start=(c == 0),
                        stop=(c == 5),
                    )
                nc.vector.tensor_scalar_add(
                    vt_sb[0:64, qs], ps[0:64, :], bias_sb[0:64, 1:2]
                )
                # V^T chunk -> 4 V' tiles (128, 64) via PE transpose
                vtp = psT.tile([128, 256], BF16, tag="vtp")
                for jj in range(4):
                    j = 4 * n + jj
                    nc.tensor.transpose(
                        vtp[:, jj * 64 : (jj + 1) * 64],
                        vt_sb[0:64, j * 128 : (j + 1) * 128],
                        ident_bf[0:64, 0:64],
                    )
                nc.vector.tensor_copy(
                    out=vv_sb[:, 4 * n : 4 * n + 4, 0:D],
                    in_=vtp[:].rearrange("p (j d) -> p j d", j=4),
                )

            def attn_pair(n, j):
                qs = slice(n * QB, (n + 1) * QB)
                sc = psS.tile([128, 1024], F32, tag="sc")
                # S^T tiles for k-tiles 2j and 2j+1, fp8 DoubleRow (0.5 cyc/row)
                for h in range(2):
                    kt = 2 * j + h
                    nc.tensor.matmul(
                        sc[:, h * 512 : (h + 1) * 512],
                        lhsT=kf_sb[:, :, kt * 128 : (kt + 1) * 128],
                        rhs=qf_sb[:, :, qs],
                        start=True,
                        stop=True,
                        perf_mode=DR,
                    )
                pt = ptp.tile([128, 1024], BF16, tag="pt")
                nc.scalar.activation(pt[:], sc[:], AF.Exp, scale=SCALE)
                pt_tiles[(n, j)] = pt

            def av_block(n):
                # flipped AV: P^T 128x128 slices stationary, V' 65-col moving.
                # sub is the OUTER loop: PSUM accumulation groups sharing one
                # bank must not interleave (hardware corrupts them otherwise).
                # Two PSUM tiles (subs 0,1 / subs 2,3) so the divides for the
                # first half overlap the second half's accumulation chains.
                oA = psOA.tile([128, 2, D + 1], F32, tag="oaccA", name=f"oaccA{rep}_{n}")
                oB = psOB.tile([128, 2, D + 1], F32, tag="oaccB", name=f"oaccB{rep}_{n}")
                oacc_tiles[n] = (oA, oB)
                for sub in range(4):
                    oacc = (oA, oB)[sub // 2]
                    for j in range(NKP):
                        pt = pt_tiles[(n, j)]
                        for h in range(2):
                            kt = 2 * j + h
                            nc.tensor.matmul(
                                oacc[:, sub % 2, :],
                                lhsT=pt[
                                    :, h * 512 + sub * 128 : h * 512 + (sub + 1) * 128
                                ],
                                rhs=vv_sb[:, kt, :],
                                start=(j == 0 and h == 0),
                                stop=(j == NKP - 1 and h == 1),
                                skip_group_check=True,
                            )

            def out_block(n):
                # per-sub divide (recip(sub) runs as soon as sub's PSUM tile
                # closes). The final block DMAs per sub to shorten the tail;
                # earlier blocks use one DMA.
                oA, oB = oacc_tiles[n]
                for sub in range(4):
                    oacc = (oA, oB)[sub // 2]
                    t = n * 4 + sub
                    rc = rcp.tile([128, 1], F32, tag="rc")
                    nc.vector.reciprocal(rc[:], oacc[:, sub % 2, D : D + 1])
                    nc.vector.tensor_scalar_mul(
                        out_sb[:, t, :], oacc[:, sub % 2, 0:D], rc[:, 0:1]
                    )
                    if n == NQB - 1:
                        nc.sync.dma_start(
                            out=out_h[t * 128 : (t + 1) * 128, :],
                            in_=out_sb[:, t, :],
                        )
                if n != NQB - 1:
                    qs = slice(n * QB, (n + 1) * QB)
                    nc.sync.dma_start(
                        out=out_h[qs, :].rearrange("(t p) i -> p t i", p=128),
                        in_=out_sb[:, n * 4 : (n + 1) * 4, :],
                    )

            # ---- software-pipelined emission, paced by chunk DMA arrival:
            # all four QK projections (+fp8 folds) first with score pairs
            # emitted as their kf/qf chunks complete, V projections deferred
            # (only needed by the AV stage), AV blocks at pt-pool turnover
            # points. reps > 1 repeats the whole computation for benchmarking.
            for rep in range(reps):
                dma_embT_chunk(0)
                if rep == 0:
                    nc.sync.dma_start(out=bias_sb[:], in_=bias_h[:])
                    nc.sync.dma_start(
                        out=wv_sb[:],
                        in_=wv_h[:].rearrange("p (c w) -> p c w", c=6),
                    )
                for n in range(1, NQB):
                    dma_embT_chunk(n)
                if rep == 0 and do_attn:
                    # PE warm-up during the DMA lead-in: dummy matmuls ramp the
                    # HAM clock gate (1.2 -> 2.4 GHz) and keep the PE busy
                    # until the first embT half lands.
                    wmm = psS.tile([128, 1024], F32, tag="sc", name="warmmm")
                    for i in range(14):
                        nc.tensor.matmul(
                            wmm[:, (i % 2) * 512 : (i % 2) * 512 + 128],
                            lhsT=ident_bf[:, 0:128],
                            rhs=ident_bf[:, 0:128],
                            start=True,
                            stop=True,
                        )
                if do_proj:
                    proj_qk_chunk(0)
                if do_attn:
                    attn_pair(0, 0)
                    attn_pair(0, 1)
                if do_proj:
                    proj_qk_chunk(1)
                if do_attn:
                    attn_pair(0, 2)
                    attn_pair(0, 3)
                    for j in range(4):
                        attn_pair(1, j)
                if do_proj:
                    proj_qk_chunk(2)
                if do_attn:
                    attn_pair(0, 4)
                    attn_pair(0, 5)
                    attn_pair(1, 4)
                    attn_pair(1, 5)
                if do_proj:
                    proj_qk_chunk(3)
                if do_attn:
                    attn_pair(0, 6)
                    attn_pair(0, 7)
                    attn_pair(1, 6)
                    attn_pair(1, 7)
                if do_proj:
                    for n in range(NQB):
                        proj_v_chunk(n)
                if do_attn:
                    for j in range(NKP):
                        attn_pair(2, j)
                    av_block(0)
                    if do_out:
                        out_block(0)
                    for j in range(4):
                        attn_pair(3, j)
                    av_block(1)
                    if do_out:
                        out_block(1)
                    for j in range(4, NKP):
                        attn_pair(3, j)
                    av_block(2)
                    if do_out:
                        out_block(2)
                    av_block(3)
                    if do_out:
                        out_block(3)
            if not do_out:
                nc.gpsimd.memset(out_sb[:, 0:1, :], 0.0)
                nc.sync.dma_start(
                    out=out_h[:].rearrange("(t p) i -> p t i", p=128),
                    in_=out_sb[:],
                )

    split_multi_waits(nc)
    return nc


_NC_CACHE = None


def _get_nc():
    global _NC_CACHE
    if _NC_CACHE is None:
        _NC_CACHE = build_nc()
    return _NC_CACHE


def make_in_maps(emb_input, Wq, bq, Wk, bk, Wv, bv):
    bf16 = ml_dtypes.bfloat16
    WqT = np.ascontiguousarray(Wq.T).astype(bf16)  # (768, 64)
    WkT = np.ascontiguousarray(Wk.T).astype(bf16)
    WvT = np.ascontiguousarray(Wv.T).astype(bf16)
    # partition-major: (6, 128, w) -> (128, 6*w) contiguous per partition
    wqk = np.concatenate([WqT, WkT], axis=1)  # (768, 128)
    wqk = np.ascontiguousarray(
        wqk.reshape(6, 128, 128).transpose(1, 0, 2).reshape(128, 768)
    )
    wv = np.ascontiguousarray(
        WvT.reshape(6, 128, 64).transpose(1, 0, 2).reshape(128, 384)
    )
    biases = np.zeros((128, 2), np.float32)
    biases[0:64, 0] = bq
    biases[0:64, 1] = bv
    in_maps = []
    for i in range(NCORES):
        embT = np.ascontiguousarray(emb_input[i].T).astype(bf16)  # (768, 2048)
        in_maps.append({"embT": embT, "wqk": wqk, "wv": wv, "biases": biases})
    return in_maps


def run(emb_input, Wq, bq, Wk, bk, Wv, bv, trace=False):
    nc = _get_nc()
    in_maps = make_in_maps(emb_input, Wq, bq, Wk, bk, Wv, bv)
    res = run_bass_kernel_spmd(nc, in_maps, core_ids=list(range(NCORES)), trace=trace)
    out = np.stack([res.results[i]["out"] for i in range(NCORES)], axis=0)
    return out.astype(np.float32), res


def kernel(emb_input, Wq, bq, Wk, bk, Wv, bv):
    out, _ = run(emb_input, Wq, bq, Wk, bk, Wv, bv, trace=False)
    return out


# revision 40
# speedup vs baseline: 1.3419x; 1.0050x over previous
"""Trainium2 Bass kernel for a single attention head.

Reference math (per batch b):
    q = emb @ Wq.T + bq ; k = emb @ Wk.T + bk ; v = emb @ Wv.T + bv
    attn = softmax((q @ k.T) / sqrt(768), axis=-1)
    out  = attn @ v

Sharding: pure data-parallel over batch. B=8 batches onto 8 NeuronCores,
one batch per core, no collectives.

Device-side layout strategy (per core):
  - emb arrives pre-transposed from the host as embT (768, 2048) bf16, one
    DMA per 512-column chunk ([128, 6, 512] tiles), so the E=768 contraction
    dim sits on SBUF partitions and HWDGE dispatch stays off the critical
    path.
  - Q and K projections are FUSED: one stationary [WqT|WkT] (768, 128) block
    computes Q^T (PSUM rows 0:64) and K^T (rows 64:128) in a single moving
    pass over each embT chunk.
  - bk is dropped: softmax over k is invariant to it.
  - Q^T/K^T are evacuated to fp8e4m3 in the DoubleRow-folded layout
    [32, 2, S] (plane j holds head dims j*32:(j+1)*32) via cross-
    partition-base DVE/Pool copies; the score matmuls then run in fp8
    DoubleRow perf mode at 0.5 cycles/row (2x bf16). Adds ~1% rel err
    (measured 1.34e-2 total vs the 2e-2 gate).
  - scores are computed transposed, S^T[k, q]; exp(S^T) (ACT, scale folded
    in) feeds the AV stage as bf16. No max-subtraction: scores*scale has
    std ~0.3, exp is safe in f32.
  - AV runs FLIPPED: P^T 128x128 slices are the stationary operand and
    V' (64 cols of V + an all-ones column for the softmax denominator Z)
    is the 65-column moving operand, accumulated over the 16 k-tiles.
    The output lands q-on-partitions, so no output transposes are needed;
    the final divide folds Z out directly from PSUM.
"""

import sys

import numpy as np

try:
    import concourse.bass as bass  # noqa: F401
except ImportError:  # pragma: no cover
    sys.path.insert(0, "/opt/trn_rl_repo")

from contextlib import ExitStack

import ml_dtypes

import concourse.bass as bass
import concourse.tile as tile
from concourse import mybir
from concourse.bass_utils import run_bass_kernel_spmd
from concourse.masks import make_identity

S = 2048  # sequence length
E = 768  # embedding dim
D = 64  # inner (head) dim
NCORES = 8
SCALE = float(1.0 / np.sqrt(np.float32(768.0)))

F32 = mybir.dt.float32
BF16 = mybir.dt.bfloat16
F8 = mybir.dt.float8e4
AF = mybir.ActivationFunctionType
DR = mybir.MatmulPerfMode.DoubleRow

QB = 512  # q block (one PSUM bank of f32)
NQB = S // QB  # 4 q blocks
NKT = S // 128  # 16 k tiles of 128
NKP = NKT // 2  # 8 k tile pairs


def split_multi_waits(nc: bass.Bass) -> int:
    """This toolchain's walrus encodes at most ONE semaphore wait per
    instruction ("Too many sync wait commands" otherwise). Tile freely emits
    multi-wait instructions, so hoist all but the last wait onto preceding
    same-engine NoOps — sequencer waits gate dispatch, so semantics are
    identical."""
    nsplit = 0
    for f in nc.m.functions:
        for bb in f.blocks:
            out = []
            changed = False
            for inst in bb.instructions:
                si = getattr(inst, "sync_info", None)
                if si is not None and len(si.on_wait) > 1:
                    waits = list(si.on_wait)
                    for w in waits[:-1]:
                        out.append(
                            mybir.InstNoOp(
                                name=nc.get_next_instruction_name(),
                                engine=inst.engine,
                                bass_nofuse=True,
                                sync_info=mybir.SyncInfo(on_wait=[w], on_update=[]),
                            )
                        )
                    inst.sync_info = mybir.SyncInfo(
                        on_wait=[waits[-1]], on_update=list(si.on_update)
                    )
                    changed = True
                    nsplit += 1
                out.append(inst)
            if changed:
                bb.instructions = out
    return nsplit


def build_nc(variant: str = "full", reps: int = 1) -> bass.Bass:
    do_proj = variant in ("full", "proj", "projattn")
    do_attn = variant in ("full", "projattn")
    do_out = variant == "full"
    nc = bass.Bass()

    embT_h = nc.declare_dram_parameter("embT", [E, S], BF16, isOutput=False)
    # host pre-arranges weights partition-major (contiguous per partition);
    # QK and V blocks are separate DMAs so the QK projections start sooner
    wqk_h = nc.declare_dram_parameter("wqk", [128, 768], BF16, isOutput=False)
    wv_h = nc.declare_dram_parameter("wv", [128, 384], BF16, isOutput=False)
    bias_h = nc.declare_dram_parameter("biases", [128, 2], F32, isOutput=False)
    out_h = nc.declare_dram_parameter("out", [S, D], F32, isOutput=True)

    with tile.TileContext(nc) as tc, ExitStack() as ctx:
        const = ctx.enter_context(tc.tile_pool(name="const", bufs=1))
        sb = ctx.enter_context(tc.tile_pool(name="sb", bufs=1))

        # ---- constants / small inputs ----
        # QK weights first: (128, 768) -> (128, 6, 128) [WqT|WkT]
        wqk_sb = const.tile([128, 6, 128], BF16, tag="wqk")
        nc.sync.dma_start(
            out=wqk_sb[:], in_=wqk_h[:].rearrange("p (c w) -> p c w", c=6)
        )
        wv_sb = const.tile([128, 6, 64], BF16, tag="wv")
        bias_sb = const.tile([128, 2], F32, tag="bias")
        ident_bf = const.tile([128, 128], BF16, tag="idbf")
        make_identity(nc, ident_bf[:])

        # warm the ACT exp table set while DMAs run
        warm = const.tile([128, 8], F32, tag="warm")
        nc.gpsimd.memset(warm[:], 0.0)
        nc.scalar.activation(warm[:], warm[:], AF.Exp)

        # ---- persistent SBUF ----
        # Q^T / K^T in fp8 zero-plane DoubleRow layout: [64, 2, S], plane 1
        # is all zeros (zeroed once on the idle ACT engine; memzero bitcasts
        # to uint32 so it is 512 elements, not 2048). The DoubleRow matmul
        # charges 0.5 cycles/row on the output columns, so the dead plane
        # costs nothing.
        qf_sb = sb.tile([64, 2, S], F8, tag="qf")
        kf_sb = sb.tile([64, 2, S], F8, tag="kf")
        nc.scalar.memzero(qf_sb[:, 1, :])
        nc.scalar.memzero(kf_sb[:, 1, :])
        vt_sb = sb.tile([64, S], BF16, tag="vt")
        # V' tiles: (k-tile, 65) with col 64 == 1.0 (softmax denominator)
        vv_sb = sb.tile([128, NKT, D + 1], BF16, tag="vv")
        nc.gpsimd.memset(vv_sb[:, :, D : D + 1], 1.0)
        out_sb = sb.tile([128, NKT, D], F32, tag="outsb")

        embT_sb = [None] * NQB

        def dma_embT_chunk(n):
            # chunk 0 arrives in three e-thirds so the first projection
            # matmuls start as soon as the first third lands; later chunks
            # use one DMA each (fewer HWDGE dispatch slots and transfer gaps)
            nparts = 3 if n == 0 else 1
            ts = []
            for part in range(nparts):
                ecs = 6 // nparts
                t = sb.tile([128, ecs, QB], BF16, tag=f"embT_{n}_{part}")
                nc.sync.dma_start(
                    out=t[:],
                    in_=embT_h[
                        part * 128 * ecs : (part + 1) * 128 * ecs,
                        n * QB : (n + 1) * QB,
                    ].rearrange("(c p) w -> p c w", p=128),
                )
                ts.append(t)
            embT_sb[n] = ts

        def embT_ap(n, c):
            ts = embT_sb[n]
            ecs = 6 // len(ts)
            return ts[c // ecs][:, c % ecs, :]

        with (
            tc.tile_pool(name="psA", bufs=1, space="PSUM") as psA,
            tc.tile_pool(name="psT", bufs=1, space="PSUM") as psT,
            tc.tile_pool(name="psOA", bufs=1, space="PSUM") as psOA,
            tc.tile_pool(name="psOB", bufs=1, space="PSUM") as psOB,
            tc.tile_pool(name="psS", bufs=2, space="PSUM") as psS,
            tc.tile_pool(name="ptp", bufs=24) as ptp,
            tc.tile_pool(name="rcp", bufs=4) as rcp,
        ):
            oacc_tiles = {}
            pt_tiles = {}

            def proj_qk_chunk(n):
                qs = slice(n * QB, (n + 1) * QB)
                ps = psA.tile([128, QB], F32, tag="proj")
                for c in range(6):
                    nc.tensor.matmul(
                        ps[:, :],
                        lhsT=wqk_sb[:, c, :],
                        rhs=embT_ap(n, c),
                        start=(c == 0),
                        stop=(c == 5),
                    )
                # evacuate to fp8 plane 0. K first: later chunks feed the
                # (0/1, j) score pairs through kf before qf is needed.
                # Chunk 0's K evac rides the still-idle ACT engine.
                if n == 0:
                    nc.scalar.copy(kf_sb[:, 0, qs], ps[64:128, :])
                else:
                    nc.vector.tensor_copy(out=kf_sb[:, 0, qs], in_=ps[64:128, :])
                nc.vector.tensor_scalar_add(
                    qf_sb[:, 0, qs], ps[0:64, :], bias_sb[0:64, 0:1]
                )

            def proj_v_chunk(n):
                qs = slice(n * QB, (n + 1) * QB)
                ps = psA.tile([128, QB], F32, tag="proj")
                for c in range(6):
                    nc.tensor.matmul(
                        ps[0:64, :],
                        lhsT=wv_sb[:, c, :],
                        rhs=embT_ap(n, c),
                        start=(c == 0),
                        stop=(c == 5),
                    )
                nc.vector.tensor_scalar_add(
                    vt_sb[0:64, qs], ps[0:64, :], bias_sb[0:64, 1:2]
                )
                # V^T chunk -> 4 V' tiles (128, 64) via PE transpose
                vtp = psT.tile([128, 256], BF16, tag="vtp")
                for jj in range(4):
                    j = 4 * n + jj
                    nc.tensor.transpose(
                        vtp[:, jj * 64 : (jj + 1) * 64],
                        vt_sb[0:64, j * 128 : (j + 1) * 128],
                        ident_bf[0:64, 0:64],
                    )
                nc.vector.tensor_copy(
                    out=vv_sb[:, 4 * n : 4 * n + 4, 0:D],
                    in_=vtp[:].rearrange("p (j d) -> p j d", j=4),
                )

            def attn_pair(n, j):
                qs = slice(n * QB, (n + 1) * QB)
                sc = psS.tile([128, 1024], F32, tag="sc")
                # S^T tiles for k-tiles 2j and 2j+1, fp8 DoubleRow (0.5 cyc/row)
                for h in range(2):
                    kt = 2 * j + h
                    nc.tensor.matmul(
                        sc[:, h * 512 : (h + 1) * 512],
                        lhsT=kf_sb[:, :, kt * 128 : (kt + 1) * 128],
                        rhs=qf_sb[:, :, qs],
                        start=True,
                        stop=True,
                        perf_mode=DR,
                    )
                pt = ptp.tile([128, 1024], BF16, tag="pt")
                nc.scalar.activation(pt[:], sc[:], AF.Exp, scale=SCALE)
                pt_tiles[(n, j)] = pt

            def av_block(n):
                # flipped AV: P^T 128x128 slices stationary, V' 65-col moving.
                # sub is the OUTER loop: PSUM accumulation groups sharing one
                # bank must not interleave (hardware corrupts them otherwise).
                # Two PSUM tiles (subs 0,1 / subs 2,3) so the divides for the
                # first half overlap the second half's accumulation chains.
                oA = psOA.tile([128, 2, D + 1], F32, tag="oaccA", name=f"oaccA{rep}_{n}")
                oB = psOB.tile([128, 2, D + 1], F32, tag="oaccB", name=f"oaccB{rep}_{n}")
                oacc_tiles[n] = (oA, oB)
                for sub in range(4):
                    oacc = (oA, oB)[sub // 2]
                    for j in range(NKP):
                        pt = pt_tiles[(n, j)]
                        for h in range(2):
                            kt = 2 * j + h
                            nc.tensor.matmul(
                                oacc[:, sub % 2, :],
                                lhsT=pt[
                                    :, h * 512 + sub * 128 : h * 512 + (sub + 1) * 128
                                ],
                                rhs=vv_sb[:, kt, :],
                                start=(j == 0 and h == 0),
                                stop=(j == NKP - 1 and h == 1),
                                skip_group_check=True,
                            )

            def out_block(n):
                # per-sub divide (recip(sub) runs as soon as sub's PSUM tile
                # closes). The final block DMAs per sub to shorten the tail;
                # earlier blocks use one DMA.
                oA, oB = oacc_tiles[n]
                for sub in range(4):
                    oacc = (oA, oB)[sub // 2]
                    t = n * 4 + sub
                    rc = rcp.tile([128, 1], F32, tag="rc")
                    nc.vector.reciprocal(rc[:], oacc[:, sub % 2, D : D + 1])
                    nc.vector.tensor_scalar_mul(
                        out_sb[:, t, :], oacc[:, sub % 2, 0:D], rc[:, 0:1]
                    )
                    if n == NQB - 1 and sub % 2 == 1:
                        # final block: DMA per sub-pair as its divides finish
                        t0 = n * 4 + sub - 1
                        nc.sync.dma_start(
                            out=out_h[t0 * 128 : (t0 + 2) * 128, :].rearrange(
                                "(t p) i -> p t i", p=128
                            ),
                            in_=out_sb[:, t0 : t0 + 2, :],
                        )
                if n != NQB - 1:
                    qs = slice(n * QB, (n + 1) * QB)
                    nc.sync.dma_start(
                        out=out_h[qs, :].rearrange("(t p) i -> p t i", p=128),
                        in_=out_sb[:, n * 4 : (n + 1) * 4, :],
                    )

            # ---- software-pipelined emission, paced by chunk DMA arrival:
            # all four QK projections (+fp8 folds) first with score pairs
            # emitted as their kf/qf chunks complete, V projections deferred
            # (only needed by the AV stage), AV blocks at pt-pool turnover
            # points. reps > 1 repeats the whole computation for benchmarking.
            for rep in range(reps):
                dma_embT_chunk(0)
                if rep == 0:
                    nc.sync.dma_start(out=bias_sb[:], in_=bias_h[:])
                    nc.sync.dma_start(
                        out=wv_sb[:],
                        in_=wv_h[:].rearrange("p (c w) -> p c w", c=6),
                    )
                for n in range(1, NQB):
                    dma_embT_chunk(n)
                if rep == 0 and do_attn:
                    # PE warm-up during the DMA lead-in: dummy matmuls ramp the
                    # HAM clock gate (1.2 -> 2.4 GHz) and keep the PE busy
                    # until the first embT half lands.
                    wmm = psS.tile([128, 1024], F32, tag="sc", name="warmmm")
                    for i in range(14):
                        nc.tensor.matmul(
                            wmm[:, (i % 2) * 512 : (i % 2) * 512 + 128],
                            lhsT=ident_bf[:, 0:128],
                            rhs=ident_bf[:, 0:128],
                            start=True,
                            stop=True,
                        )
                if do_proj:
                    proj_qk_chunk(0)
                if do_attn:
                    attn_pair(0, 0)
                    attn_pair(0, 1)
                if do_proj:
                    proj_qk_chunk(1)
                if do_attn:
                    attn_pair(0, 2)
                    attn_pair(0, 3)
                    for j in range(4):
                        attn_pair(1, j)
                if do_proj:
                    proj_qk_chunk(2)
                if do_attn:
                    attn_pair(0, 4)
                    attn_pair(0, 5)
                    attn_pair(1, 4)
                    attn_pair(1, 5)
                if do_proj:
                    proj_qk_chunk(3)
                if do_attn:
                    attn_pair(0, 6)
                    attn_pair(0, 7)
                    attn_pair(1, 6)
                    attn_pair(1, 7)
                if do_proj:
                    for n in range(NQB):
                        proj_v_chunk(n)
                if do_attn:
                    for j in range(NKP):
                        attn_pair(2, j)
                    av_block(0)
                    if do_out:
                        out_block(0)
                    for j in range(4):
                        attn_pair(3, j)
                    av_block(1)
                    if do_out:
                        out_block(1)
                    for j in range(4, NKP):
                        attn_pair(3, j)
                    av_block(2)
                    if do_out:
                        out_block(2)
                    av_block(3)
                    if do_out:
                        out_block(3)
            if not do_out:
                nc.gpsimd.memset(out_sb[:, 0:1, :], 0.0)
                nc.sync.dma_start(
                    out=out_h[:].rearrange("(t p) i -> p t i", p=128),
                    in_=out_sb[:],
                )

    split_multi_waits(nc)
    return nc


_NC_CACHE = None


def _get_nc():
    global _NC_CACHE
    if _NC_CACHE is None:
        _NC_CACHE = build_nc()
    return _NC_CACHE


def make_in_maps(emb_input, Wq, bq, Wk, bk, Wv, bv):
    bf16 = ml_dtypes.bfloat16
    WqT = np.ascontiguousarray(Wq.T).astype(bf16)  # (768, 64)
    WkT = np.ascontiguousarray(Wk.T).astype(bf16)
    WvT = np.ascontiguousarray(Wv.T).astype(bf16)
    # partition-major: (6, 128, w) -> (128, 6*w) contiguous per partition
    wqk = np.concatenate([WqT, WkT], axis=1)  # (768, 128)
    wqk = np.ascontiguousarray(
        wqk.reshape(6, 128, 128).transpose(1, 0, 2).reshape(128, 768)
    )
    wv = np.ascontiguousarray(
        WvT.reshape(6, 128, 64).transpose(1, 0, 2).reshape(128, 384)
    )
    biases = np.zeros((128, 2), np.float32)
    biases[0:64, 0] = bq
    biases[0:64, 1] = bv
    in_maps = []
    for i in range(NCORES):
        embT = np.ascontiguousarray(emb_input[i].T).astype(bf16)  # (768, 2048)
        in_maps.append({"embT": embT, "wqk": wqk, "wv": wv, "biases": biases})
    return in_maps


def run(emb_input, Wq, bq, Wk, bk, Wv, bv, trace=False):
    nc = _get_nc()
    in_maps = make_in_maps(emb_input, Wq, bq, Wk, bk, Wv, bv)
    res = run_bass_kernel_spmd(nc, in_maps, core_ids=list(range(NCORES)), trace=trace)
    out = np.stack([res.results[i]["out"] for i in range(NCORES)], axis=0)
    return out.astype(np.float32), res


def kernel(emb_input, Wq, bq, Wk, bk, Wv, bv):
    out, _ = run(emb_input, Wq, bq, Wk, bk, Wv, bv, trace=False)
    return out


# revision 42
# speedup vs baseline: 1.3569x; 1.0112x over previous
"""Trainium2 Bass kernel for a single attention head.

Reference math (per batch b):
    q = emb @ Wq.T + bq ; k = emb @ Wk.T + bk ; v = emb @ Wv.T + bv
    attn = softmax((q @ k.T) / sqrt(768), axis=-1)
    out  = attn @ v

Sharding: pure data-parallel over batch. B=8 batches onto 8 NeuronCores,
one batch per core, no collectives.

Device-side layout strategy (per core):
  - emb arrives pre-transposed from the host as embT (768, 2048) bf16, one
    DMA per 512-column chunk ([128, 6, 512] tiles), so the E=768 contraction
    dim sits on SBUF partitions and HWDGE dispatch stays off the critical
    path.
  - Q and K projections are FUSED: one stationary [WqT|WkT] (768, 128) block
    computes Q^T (PSUM rows 0:64) and K^T (rows 64:128) in a single moving
    pass over each embT chunk.
  - bk is dropped: softmax over k is invariant to it.
  - Q^T/K^T are evacuated to fp8e4m3 in the DoubleRow-folded layout
    [32, 2, S] (plane j holds head dims j*32:(j+1)*32) via cross-
    partition-base DVE/Pool copies; the score matmuls then run in fp8
    DoubleRow perf mode at 0.5 cycles/row (2x bf16). Adds ~1% rel err
    (measured 1.34e-2 total vs the 2e-2 gate).
  - scores are computed transposed, S^T[k, q]; exp(S^T) (ACT, scale folded
    in) feeds the AV stage as bf16. No max-subtraction: scores*scale has
    std ~0.3, exp is safe in f32.
  - AV runs FLIPPED: P^T 128x128 slices are the stationary operand and
    V' (64 cols of V + an all-ones column for the softmax denominator Z)
    is the 65-column moving operand, accumulated over the 16 k-tiles.
    The output lands q-on-partitions, so no output transposes are needed;
    the final divide folds Z out directly from PSUM.
"""

import sys

import numpy as np

try:
    import concourse.bass as bass  # noqa: F401
except ImportError:  # pragma: no cover
    sys.path.insert(0, "/opt/trn_rl_repo")

from contextlib import ExitStack

import ml_dtypes

import concourse.bass as bass
import concourse.tile as tile
from concourse import mybir
from concourse.bass_utils import run_bass_kernel_spmd
from concourse.masks import make_identity

S = 2048  # sequence length
E = 768  # embedding dim
D = 64  # inner (head) dim
NCORES = 8
SCALE = float(1.0 / np.sqrt(np.float32(768.0)))

F32 = mybir.dt.float32
BF16 = mybir.dt.bfloat16
F8 = mybir.dt.float8e4
AF = mybir.ActivationFunctionType
DR = mybir.MatmulPerfMode.DoubleRow

QB = 512  # q block (one PSUM bank of f32)
NQB = S // QB  # 4 q blocks
NKT = S // 128  # 16 k tiles of 128
NKP = NKT // 2  # 8 k tile pairs


def split_multi_waits(nc: bass.Bass) -> int:
    """This toolchain's walrus encodes at most ONE semaphore wait per
    instruction ("Too many sync wait commands" otherwise). Tile freely emits
    multi-wait instructions, so hoist all but the last wait onto preceding
    same-engine NoOps — sequencer waits gate dispatch, so semantics are
    identical."""
    nsplit = 0
    for f in nc.m.functions:
        for bb in f.blocks:
            out = []
            changed = False
            for inst in bb.instructions:
                si = getattr(inst, "sync_info", None)
                if si is not None and len(si.on_wait) > 1:
                    waits = list(si.on_wait)
                    for w in waits[:-1]:
                        out.append(
                            mybir.InstNoOp(
                                name=nc.get_next_instruction_name(),
                                engine=inst.engine,
                                bass_nofuse=True,
                                sync_info=mybir.SyncInfo(on_wait=[w], on_update=[]),
                            )
                        )
                    inst.sync_info = mybir.SyncInfo(
                        on_wait=[waits[-1]], on_update=list(si.on_update)
                    )
                    changed = True
                    nsplit += 1
                out.append(inst)
            if changed:
                bb.instructions = out
    return nsplit


def build_nc(variant: str = "full", reps: int = 1) -> bass.Bass:
    do_proj = variant in ("full", "proj", "projattn")
    do_attn = variant in ("full", "projattn")
    do_out = variant == "full"
    nc = bass.Bass()

    embT_h = nc.declare_dram_parameter("embT", [E, S], BF16, isOutput=False)
    # host pre-arranges weights partition-major (contiguous per partition);
    # QK and V blocks are separate DMAs so the QK projections start sooner
    wqk_h = nc.declare_dram_parameter("wqk", [128, 768], BF16, isOutput=False)
    wv_h = nc.declare_dram_parameter("wv", [128, 384], BF16, isOutput=False)
    bias_h = nc.declare_dram_parameter("biases", [128, 2], F32, isOutput=False)
    out_h = nc.declare_dram_parameter("out", [S, D], F32, isOutput=True)

    with tile.TileContext(nc) as tc, ExitStack() as ctx:
        const = ctx.enter_context(tc.tile_pool(name="const", bufs=1))
        sb = ctx.enter_context(tc.tile_pool(name="sb", bufs=1))

        # ---- constants / small inputs ----
        # QK weights first: (128, 768) -> (128, 6, 128) [WqT|WkT]
        wqk_sb = const.tile([128, 6, 128], BF16, tag="wqk")
        nc.sync.dma_start(
            out=wqk_sb[:], in_=wqk_h[:].rearrange("p (c w) -> p c w", c=6)
        )
        wv_sb = const.tile([128, 6, 64], BF16, tag="wv")
        bias_sb = const.tile([128, 2], F32, tag="bias")
        ident_bf = const.tile([128, 128], BF16, tag="idbf")
        make_identity(nc, ident_bf[:])

        # warm the ACT exp table set while DMAs run
        warm = const.tile([128, 8], F32, tag="warm")
        nc.gpsimd.memset(warm[:], 0.0)
        nc.scalar.activation(warm[:], warm[:], AF.Exp)

        # ---- persistent SBUF ----
        # Q^T / K^T in fp8 zero-plane DoubleRow layout: [64, 2, S], plane 1
        # is all zeros (zeroed once on the idle ACT engine; memzero bitcasts
        # to uint32 so it is 512 elements, not 2048). The DoubleRow matmul
        # charges 0.5 cycles/row on the output columns, so the dead plane
        # costs nothing.
        qf_sb = sb.tile([64, 2, S], F8, tag="qf")
        kf_sb = sb.tile([64, 2, S], F8, tag="kf")
        nc.scalar.memzero(qf_sb[:, 1, :])
        nc.scalar.memzero(kf_sb[:, 1, :])
        vt_sb = sb.tile([64, S], BF16, tag="vt")
        # V' tiles: (k-tile, 65) with col 64 == 1.0 (softmax denominator)
        vv_sb = sb.tile([128, NKT, D + 1], BF16, tag="vv")
        nc.gpsimd.memset(vv_sb[:, :, D : D + 1], 1.0)
        out_sb = sb.tile([128, NKT, D], F32, tag="outsb")

        embT_sb = [None] * NQB

        def dma_embT_chunk(n):
            # chunk 0 arrives in three e-thirds so the first projection
            # matmuls start as soon as the first third lands; chunk 1 in two
            # halves (its kf gates the exp stream early); later chunks use
            # one DMA each (fewer HWDGE dispatch slots and transfer gaps)
            nparts = 3 if n == 0 else (2 if n == 1 else 1)
            ts = []
            for part in range(nparts):
                ecs = 6 // nparts
                t = sb.tile([128, ecs, QB], BF16, tag=f"embT_{n}_{part}")
                nc.sync.dma_start(
                    out=t[:],
                    in_=embT_h[
                        part * 128 * ecs : (part + 1) * 128 * ecs,
                        n * QB : (n + 1) * QB,
                    ].rearrange("(c p) w -> p c w", p=128),
                )
                ts.append(t)
            embT_sb[n] = ts

        def embT_ap(n, c):
            ts = embT_sb[n]
            ecs = 6 // len(ts)
            return ts[c // ecs][:, c % ecs, :]

        with (
            tc.tile_pool(name="psA", bufs=1, space="PSUM") as psA,
            tc.tile_pool(name="psT", bufs=1, space="PSUM") as psT,
            tc.tile_pool(name="psOA", bufs=1, space="PSUM") as psOA,
            tc.tile_pool(name="psOB", bufs=1, space="PSUM") as psOB,
            tc.tile_pool(name="psS", bufs=2, space="PSUM") as psS,
            tc.tile_pool(name="ptp", bufs=24) as ptp,
            tc.tile_pool(name="rcp", bufs=4) as rcp,
        ):
            oacc_tiles = {}
            pt_tiles = {}

            def proj_qk_chunk(n):
                qs = slice(n * QB, (n + 1) * QB)
                ps = psA.tile([128, QB], F32, tag="proj")
                for c in range(6):
                    nc.tensor.matmul(
                        ps[:, :],
                        lhsT=wqk_sb[:, c, :],
                        rhs=embT_ap(n, c),
                        start=(c == 0),
                        stop=(c == 5),
                    )
                # evacuate to fp8 plane 0. K first: later chunks feed the
                # (0/1, j) score pairs through kf before qf is needed.
                # Chunk 0's K evac rides the still-idle ACT engine.
                if n == 0:
                    nc.vector.tensor_scalar_add(
                        qf_sb[:, 0, qs], ps[0:64, :], bias_sb[0:64, 0:1]
                    )
                    nc.scalar.copy(kf_sb[:, 0, qs], ps[64:128, :])
                else:
                    nc.vector.tensor_copy(out=kf_sb[:, 0, qs], in_=ps[64:128, :])
                    nc.vector.tensor_scalar_add(
                        qf_sb[:, 0, qs], ps[0:64, :], bias_sb[0:64, 0:1]
                    )

            def proj_v_chunk(n):
                qs = slice(n * QB, (n + 1) * QB)
                ps = psA.tile([128, QB], F32, tag="proj")
                for c in range(6):
                    nc.tensor.matmul(
                        ps[0:64, :],
                        lhsT=wv_sb[:, c, :],
                        rhs=embT_ap(n, c),
                        start=(c == 0),
                        stop=(c == 5),
                    )
                nc.vector.tensor_scalar_add(
                    vt_sb[0:64, qs], ps[0:64, :], bias_sb[0:64, 1:2]
                )
                # V^T chunk -> 4 V' tiles (128, 64) via PE transpose
                vtp = psT.tile([128, 256], BF16, tag="vtp")
                for jj in range(4):
                    j = 4 * n + jj
                    nc.tensor.transpose(
                        vtp[:, jj * 64 : (jj + 1) * 64],
                        vt_sb[0:64, j * 128 : (j + 1) * 128],
                        ident_bf[0:64, 0:64],
                    )
                nc.vector.tensor_copy(
                    out=vv_sb[:, 4 * n : 4 * n + 4, 0:D],
                    in_=vtp[:].rearrange("p (j d) -> p j d", j=4),
                )

            def attn_pair(n, j):
                qs = slice(n * QB, (n + 1) * QB)
                sc = psS.tile([128, 1024], F32, tag="sc")
                # S^T tiles for k-tiles 2j and 2j+1, fp8 DoubleRow (0.5 cyc/row)
                for h in range(2):
                    kt = 2 * j + h
                    nc.tensor.matmul(
                        sc[:, h * 512 : (h + 1) * 512],
                        lhsT=kf_sb[:, :, kt * 128 : (kt + 1) * 128],
                        rhs=qf_sb[:, :, qs],
                        start=True,
                        stop=True,
                        perf_mode=DR,
                    )
                pt = ptp.tile([128, 1024], BF16, tag="pt")
                nc.scalar.activation(pt[:], sc[:], AF.Exp, scale=SCALE)
                pt_tiles[(n, j)] = pt

            def av_block(n):
                # flipped AV: P^T 128x128 slices stationary, V' 65-col moving.
                # sub is the OUTER loop: PSUM accumulation groups sharing one
                # bank must not interleave (hardware corrupts them otherwise).
                # Two PSUM tiles (subs 0,1 / subs 2,3) so the divides for the
                # first half overlap the second half's accumulation chains.
                oA = psOA.tile([128, 2, D + 1], F32, tag="oaccA", name=f"oaccA{rep}_{n}")
                oB = psOB.tile([128, 2, D + 1], F32, tag="oaccB", name=f"oaccB{rep}_{n}")
                oacc_tiles[n] = (oA, oB)
                for sub in range(4):
                    oacc = (oA, oB)[sub // 2]
                    for j in range(NKP):
                        pt = pt_tiles[(n, j)]
                        for h in range(2):
                            kt = 2 * j + h
                            nc.tensor.matmul(
                                oacc[:, sub % 2, :],
                                lhsT=pt[
                                    :, h * 512 + sub * 128 : h * 512 + (sub + 1) * 128
                                ],
                                rhs=vv_sb[:, kt, :],
                                start=(j == 0 and h == 0),
                                stop=(j == NKP - 1 and h == 1),
                                skip_group_check=True,
                            )

            def out_block(n):
                # per-sub divide (recip(sub) runs as soon as sub's PSUM tile
                # closes). The final block DMAs per sub to shorten the tail;
                # earlier blocks use one DMA.
                oA, oB = oacc_tiles[n]
                for sub in range(4):
                    oacc = (oA, oB)[sub // 2]
                    t = n * 4 + sub
                    rc = rcp.tile([128, 1], F32, tag="rc")
                    nc.vector.reciprocal(rc[:], oacc[:, sub % 2, D : D + 1])
                    nc.vector.tensor_scalar_mul(
                        out_sb[:, t, :], oacc[:, sub % 2, 0:D], rc[:, 0:1]
                    )
                    if n == NQB - 1 and sub % 2 == 1:
                        # final block: DMA per sub-pair as its divides finish
                        t0 = n * 4 + sub - 1
                        nc.sync.dma_start(
                            out=out_h[t0 * 128 : (t0 + 2) * 128, :].rearrange(
                                "(t p) i -> p t i", p=128
                            ),
                            in_=out_sb[:, t0 : t0 + 2, :],
                        )
                if n != NQB - 1:
                    qs = slice(n * QB, (n + 1) * QB)
                    nc.sync.dma_start(
                        out=out_h[qs, :].rearrange("(t p) i -> p t i", p=128),
                        in_=out_sb[:, n * 4 : (n + 1) * 4, :],
                    )

            # ---- software-pipelined emission, paced by chunk DMA arrival:
            # all four QK projections (+fp8 folds) first with score pairs
            # emitted as their kf/qf chunks complete, V projections deferred
            # (only needed by the AV stage), AV blocks at pt-pool turnover
            # points. reps > 1 repeats the whole computation for benchmarking.
            for rep in range(reps):
                dma_embT_chunk(0)
                if rep == 0:
                    nc.sync.dma_start(out=bias_sb[:], in_=bias_h[:])
                    nc.sync.dma_start(
                        out=wv_sb[:],
                        in_=wv_h[:].rearrange("p (c w) -> p c w", c=6),
                    )
                for n in range(1, NQB):
                    dma_embT_chunk(n)
                if rep == 0 and do_attn:
                    # PE warm-up during the DMA lead-in: dummy matmuls ramp the
                    # HAM clock gate (1.2 -> 2.4 GHz) and keep the PE busy
                    # until the first embT half lands.
                    wmm = psS.tile([128, 1024], F32, tag="sc", name="warmmm")
                    for i in range(14):
                        nc.tensor.matmul(
                            wmm[:, (i % 2) * 512 : (i % 2) * 512 + 128],
                            lhsT=ident_bf[:, 0:128],
                            rhs=ident_bf[:, 0:128],
                            start=True,
                            stop=True,
                        )
                if do_proj:
                    proj_qk_chunk(0)
                if do_attn:
                    attn_pair(0, 0)
                    attn_pair(0, 1)
                if do_proj:
                    proj_qk_chunk(1)
                if do_attn:
                    attn_pair(0, 2)
                    attn_pair(0, 3)
                    for j in range(4):
                        attn_pair(1, j)
                if do_proj:
                    proj_qk_chunk(2)
                if do_attn:
                    attn_pair(0, 4)
                    attn_pair(0, 5)
                    attn_pair(1, 4)
                    attn_pair(1, 5)
                if do_proj:
                    proj_qk_chunk(3)
                if do_attn:
                    attn_pair(0, 6)
                    attn_pair(0, 7)
                    attn_pair(1, 6)
                    attn_pair(1, 7)
                if do_proj:
                    for n in range(NQB):
                        proj_v_chunk(n)
                if do_attn:
                    for j in range(NKP):
                        attn_pair(2, j)
                    av_block(0)
                    if do_out:
                        out_block(0)
                    for j in range(4):
                        attn_pair(3, j)
                    av_block(1)
                    if do_out:
                        out_block(1)
                    for j in range(4, NKP):
                        attn_pair(3, j)
                    av_block(2)
                    if do_out:
                        out_block(2)
                    av_block(3)
                    if do_out:
                        out_block(3)
            if not do_out:
                nc.gpsimd.memset(out_sb[:, 0:1, :], 0.0)
                nc.sync.dma_start(
                    out=out_h[:].rearrange("(t p) i -> p t i", p=128),
                    in_=out_sb[:],
                )

    split_multi_waits(nc)
    return nc


_NC_CACHE = None


def _get_nc():
    global _NC_CACHE
    if _NC_CACHE is None:
        _NC_CACHE = build_nc()
    return _NC_CACHE


def make_in_maps(emb_input, Wq, bq, Wk, bk, Wv, bv):
    bf16 = ml_dtypes.bfloat16
    WqT = np.ascontiguousarray(Wq.T).astype(bf16)  # (768, 64)
    WkT = np.ascontiguousarray(Wk.T).astype(bf16)
    WvT = np.ascontiguousarray(Wv.T).astype(bf16)
    # partition-major: (6, 128, w) -> (128, 6*w) contiguous per partition
    wqk = np.concatenate([WqT, WkT], axis=1)  # (768, 128)
    wqk = np.ascontiguousarray(
        wqk.reshape(6, 128, 128).transpose(1, 0, 2).reshape(128, 768)
    )
    wv = np.ascontiguousarray(
        WvT.reshape(6, 128, 64).transpose(1, 0, 2).reshape(128, 384)
    )
    biases = np.zeros((128, 2), np.float32)
    biases[0:64, 0] = bq
    biases[0:64, 1] = bv
    in_maps = []
    for i in range(NCORES):
        embT = np.ascontiguousarray(emb_input[i].T).astype(bf16)  # (768, 2048)
        in_maps.append({"embT": embT, "wqk": wqk, "wv": wv, "biases": biases})
    return in_maps


def run(emb_input, Wq, bq, Wk, bk, Wv, bv, trace=False):
    nc = _get_nc()
    in_maps = make_in_maps(emb_input, Wq, bq, Wk, bk, Wv, bv)
    res = run_bass_kernel_spmd(nc, in_maps, core_ids=list(range(NCORES)), trace=trace)
    out = np.stack([res.results[i]["out"] for i in range(NCORES)], axis=0)
    return out.astype(np.float32), res


def kernel(emb_input, Wq, bq, Wk, bk, Wv, bv):
    out, _ = run(emb_input, Wq, bq, Wk, bk, Wv, bv, trace=False)
    return out


# revision 43
# speedup vs baseline: 1.3770x; 1.0148x over previous
"""Trainium2 Bass kernel for a single attention head.

Reference math (per batch b):
    q = emb @ Wq.T + bq ; k = emb @ Wk.T + bk ; v = emb @ Wv.T + bv
    attn = softmax((q @ k.T) / sqrt(768), axis=-1)
    out  = attn @ v

Sharding: pure data-parallel over batch. B=8 batches onto 8 NeuronCores,
one batch per core, no collectives.

Device-side layout strategy (per core):
  - emb arrives pre-transposed from the host as embT (768, 2048) bf16, one
    DMA per 512-column chunk ([128, 6, 512] tiles), so the E=768 contraction
    dim sits on SBUF partitions and HWDGE dispatch stays off the critical
    path.
  - Q and K projections are FUSED: one stationary [WqT|WkT] (768, 128) block
    computes Q^T (PSUM rows 0:64) and K^T (rows 64:128) in a single moving
    pass over each embT chunk.
  - bk is dropped: softmax over k is invariant to it.
  - Q^T/K^T are evacuated to fp8e4m3 in the DoubleRow-folded layout
    [32, 2, S] (plane j holds head dims j*32:(j+1)*32) via cross-
    partition-base DVE/Pool copies; the score matmuls then run in fp8
    DoubleRow perf mode at 0.5 cycles/row (2x bf16). Adds ~1% rel err
    (measured 1.34e-2 total vs the 2e-2 gate).
  - scores are computed transposed, S^T[k, q]; exp(S^T) (ACT, scale folded
    in) feeds the AV stage as bf16. No max-subtraction: scores*scale has
    std ~0.3, exp is safe in f32.
  - AV runs FLIPPED: P^T 128x128 slices are the stationary operand and
    V' (64 cols of V + an all-ones column for the softmax denominator Z)
    is the 65-column moving operand, accumulated over the 16 k-tiles.
    The output lands q-on-partitions, so no output transposes are needed;
    the final divide folds Z out directly from PSUM.
"""

import sys

import numpy as np

try:
    import concourse.bass as bass  # noqa: F401
except ImportError:  # pragma: no cover
    sys.path.insert(0, "/opt/trn_rl_repo")

from contextlib import ExitStack

import ml_dtypes

import concourse.bass as bass
import concourse.tile as tile
from concourse import mybir
from concourse.bass_utils import run_bass_kernel_spmd
from concourse.masks import make_identity

S = 2048  # sequence length
E = 768  # embedding dim
D = 64  # inner (head) dim
NCORES = 8
SCALE = float(1.0 / np.sqrt(np.float32(768.0)))

F32 = mybir.dt.float32
BF16 = mybir.dt.bfloat16
F8 = mybir.dt.float8e4
AF = mybir.ActivationFunctionType
DR = mybir.MatmulPerfMode.DoubleRow

QB = 512  # q block (one PSUM bank of f32)
NQB = S // QB  # 4 q blocks
NKT = S // 128  # 16 k tiles of 128
NKP = NKT // 2  # 8 k tile pairs


def split_multi_waits(nc: bass.Bass) -> int:
    """This toolchain's walrus encodes at most ONE semaphore wait per
    instruction ("Too many sync wait commands" otherwise). Tile freely emits
    multi-wait instructions, so hoist all but the last wait onto preceding
    same-engine NoOps — sequencer waits gate dispatch, so semantics are
    identical."""
    nsplit = 0
    for f in nc.m.functions:
        for bb in f.blocks:
            out = []
            changed = False
            for inst in bb.instructions:
                si = getattr(inst, "sync_info", None)
                if si is not None and len(si.on_wait) > 1:
                    waits = list(si.on_wait)
                    for w in waits[:-1]:
                        out.append(
                            mybir.InstNoOp(
                                name=nc.get_next_instruction_name(),
                                engine=inst.engine,
                                bass_nofuse=True,
                                sync_info=mybir.SyncInfo(on_wait=[w], on_update=[]),
                            )
                        )
                    inst.sync_info = mybir.SyncInfo(
                        on_wait=[waits[-1]], on_update=list(si.on_update)
                    )
                    changed = True
                    nsplit += 1
                out.append(inst)
            if changed:
                bb.instructions = out
    return nsplit


def build_nc(variant: str = "full", reps: int = 1) -> bass.Bass:
    do_proj = variant in ("full", "proj", "projattn")
    do_attn = variant in ("full", "projattn")
    do_out = variant == "full"
    nc = bass.Bass()

    embT_h = nc.declare_dram_parameter("embT", [E, S], BF16, isOutput=False)
    # host pre-arranges weights partition-major (contiguous per partition);
    # QK and V blocks are separate DMAs so the QK projections start sooner
    wqk_h = nc.declare_dram_parameter("wqk", [128, 768], BF16, isOutput=False)
    wv_h = nc.declare_dram_parameter("wv", [128, 384], BF16, isOutput=False)
    bias_h = nc.declare_dram_parameter("biases", [128, 2], F32, isOutput=False)
    out_h = nc.declare_dram_parameter("out", [S, D], F32, isOutput=True)

    with tile.TileContext(nc) as tc, ExitStack() as ctx:
        const = ctx.enter_context(tc.tile_pool(name="const", bufs=1))
        sb = ctx.enter_context(tc.tile_pool(name="sb", bufs=1))

        # ---- constants / small inputs ----
        # QK weights first: (128, 768) -> (128, 6, 128) [WqT|WkT]
        wqk_sb = const.tile([128, 6, 128], BF16, tag="wqk")
        nc.sync.dma_start(
            out=wqk_sb[:], in_=wqk_h[:].rearrange("p (c w) -> p c w", c=6)
        )
        wv_sb = const.tile([128, 6, 64], BF16, tag="wv")
        bias_sb = const.tile([128, 2], F32, tag="bias")
        ident_bf = const.tile([128, 128], BF16, tag="idbf")
        make_identity(nc, ident_bf[:])

        # warm the ACT exp table set while DMAs run
        warm = const.tile([128, 8], F32, tag="warm")
        nc.gpsimd.memset(warm[:], 0.0)
        nc.scalar.activation(warm[:], warm[:], AF.Exp)

        # ---- persistent SBUF ----
        # Q^T / K^T in fp8 zero-plane DoubleRow layout: [64, 2, S], plane 1
        # is all zeros (zeroed once on the idle ACT engine; memzero bitcasts
        # to uint32 so it is 512 elements, not 2048). The DoubleRow matmul
        # charges 0.5 cycles/row on the output columns, so the dead plane
        # costs nothing.
        qf_sb = sb.tile([64, 2, S], F8, tag="qf")
        kf_sb = sb.tile([64, 2, S], F8, tag="kf")
        nc.scalar.memzero(qf_sb[:, 1, :])
        nc.scalar.memzero(kf_sb[:, 1, :])
        vt_sb = sb.tile([64, S], BF16, tag="vt")
        # V' tiles: (k-tile, 65) with col 64 == 1.0 (softmax denominator)
        vv_sb = sb.tile([128, NKT, D + 1], BF16, tag="vv")
        nc.gpsimd.memset(vv_sb[:, :, D : D + 1], 1.0)
        out_sb = sb.tile([128, NKT, D], F32, tag="outsb")

        embT_sb = [None] * NQB

        def dma_embT_chunk(n):
            # chunk 0 arrives in three e-thirds so the first projection
            # matmuls start as soon as the first third lands; chunk 1 in two
            # halves (its kf gates the exp stream early); later chunks use
            # one DMA each (fewer HWDGE dispatch slots and transfer gaps)
            nparts = 3 if n == 0 else (2 if n == 1 else 1)
            ts = []
            for part in range(nparts):
                ecs = 6 // nparts
                t = sb.tile([128, ecs, QB], BF16, tag=f"embT_{n}_{part}")
                nc.sync.dma_start(
                    out=t[:],
                    in_=embT_h[
                        part * 128 * ecs : (part + 1) * 128 * ecs,
                        n * QB : (n + 1) * QB,
                    ].rearrange("(c p) w -> p c w", p=128),
                )
                ts.append(t)
            embT_sb[n] = ts

        def embT_ap(n, c):
            ts = embT_sb[n]
            ecs = 6 // len(ts)
            return ts[c // ecs][:, c % ecs, :]

        with (
            tc.tile_pool(name="psA", bufs=1, space="PSUM") as psA,
            tc.tile_pool(name="psT", bufs=1, space="PSUM") as psT,
            tc.tile_pool(name="psOA", bufs=1, space="PSUM") as psOA,
            tc.tile_pool(name="psOB", bufs=1, space="PSUM") as psOB,
            tc.tile_pool(name="psS", bufs=2, space="PSUM") as psS,
            tc.tile_pool(name="ptp", bufs=24) as ptp,
            tc.tile_pool(name="rcp", bufs=4) as rcp,
        ):
            oacc_tiles = {}
            pt_tiles = {}

            def proj_qk_chunk(n):
                qs = slice(n * QB, (n + 1) * QB)
                ps = psA.tile([128, QB], F32, tag="proj")
                for c in range(6):
                    nc.tensor.matmul(
                        ps[:, :],
                        lhsT=wqk_sb[:, c, :],
                        rhs=embT_ap(n, c),
                        start=(c == 0),
                        stop=(c == 5),
                    )
                # evacuate to fp8 plane 0. K first: later chunks feed the
                # (0/1, j) score pairs through kf before qf is needed.
                # Chunk 0's K evac rides the still-idle ACT engine.
                if n == 0:
                    nc.vector.tensor_scalar_add(
                        qf_sb[:, 0, qs], ps[0:64, :], bias_sb[0:64, 0:1]
                    )
                    nc.scalar.copy(kf_sb[:, 0, qs], ps[64:128, :])
                else:
                    nc.vector.tensor_copy(out=kf_sb[:, 0, qs], in_=ps[64:128, :])
                    nc.vector.tensor_scalar_add(
                        qf_sb[:, 0, qs], ps[0:64, :], bias_sb[0:64, 0:1]
                    )

            def proj_v_chunk(n):
                qs = slice(n * QB, (n + 1) * QB)
                ps = psA.tile([128, QB], F32, tag="proj")
                for c in range(6):
                    nc.tensor.matmul(
                        ps[0:64, :],
                        lhsT=wv_sb[:, c, :],
                        rhs=embT_ap(n, c),
                        start=(c == 0),
                        stop=(c == 5),
                    )
                nc.vector.tensor_scalar_add(
                    vt_sb[0:64, qs], ps[0:64, :], bias_sb[0:64, 1:2]
                )
                # V^T chunk -> 4 V' tiles (128, 64) via PE transpose
                vtp = psT.tile([128, 256], BF16, tag="vtp")
                for jj in range(4):
                    j = 4 * n + jj
                    nc.tensor.transpose(
                        vtp[:, jj * 64 : (jj + 1) * 64],
                        vt_sb[0:64, j * 128 : (j + 1) * 128],
                        ident_bf[0:64, 0:64],
                    )
                nc.vector.tensor_copy(
                    out=vv_sb[:, 4 * n : 4 * n + 4, 0:D],
                    in_=vtp[:].rearrange("p (j d) -> p j d", j=4),
                )

            def attn_pair(n, j):
                qs = slice(n * QB, (n + 1) * QB)
                sc = psS.tile([128, 1024], F32, tag="sc")
                # S^T tiles for k-tiles 2j and 2j+1, fp8 DoubleRow (0.5 cyc/row)
                for h in range(2):
                    kt = 2 * j + h
                    nc.tensor.matmul(
                        sc[:, h * 512 : (h + 1) * 512],
                        lhsT=kf_sb[:, :, kt * 128 : (kt + 1) * 128],
                        rhs=qf_sb[:, :, qs],
                        start=True,
                        stop=True,
                        perf_mode=DR,
                    )
                pt = ptp.tile([128, 1024], BF16, tag="pt")
                nc.scalar.activation(pt[:], sc[:], AF.Exp, scale=SCALE)
                pt_tiles[(n, j)] = pt

            def av_block(n):
                # flipped AV: P^T 128x128 slices stationary, V' 65-col moving.
                # sub is the OUTER loop: PSUM accumulation groups sharing one
                # bank must not interleave (hardware corrupts them otherwise).
                # Two PSUM tiles (subs 0,1 / subs 2,3) so the divides for the
                # first half overlap the second half's accumulation chains.
                oA = psOA.tile([128, 2, D + 1], F32, tag="oaccA", name=f"oaccA{rep}_{n}")
                oB = psOB.tile([128, 2, D + 1], F32, tag="oaccB", name=f"oaccB{rep}_{n}")
                oacc_tiles[n] = (oA, oB)
                for sub in range(4):
                    oacc = (oA, oB)[sub // 2]
                    for j in range(NKP):
                        pt = pt_tiles[(n, j)]
                        for h in range(2):
                            kt = 2 * j + h
                            nc.tensor.matmul(
                                oacc[:, sub % 2, :],
                                lhsT=pt[
                                    :, h * 512 + sub * 128 : h * 512 + (sub + 1) * 128
                                ],
                                rhs=vv_sb[:, kt, :],
                                start=(j == 0 and h == 0),
                                stop=(j == NKP - 1 and h == 1),
                                skip_group_check=True,
                            )

            def out_block(n):
                # per-sub divide (recip(sub) runs as soon as sub's PSUM tile
                # closes). The final block DMAs per sub to shorten the tail;
                # earlier blocks use one DMA.
                oA, oB = oacc_tiles[n]
                for sub in range(4):
                    oacc = (oA, oB)[sub // 2]
                    t = n * 4 + sub
                    rc = rcp.tile([128, 1], F32, tag="rc")
                    nc.vector.reciprocal(rc[:], oacc[:, sub % 2, D : D + 1])
                    nc.vector.tensor_scalar_mul(
                        out_sb[:, t, :], oacc[:, sub % 2, 0:D], rc[:, 0:1]
                    )
                    if n == NQB - 1 and sub % 2 == 1:
                        # final block: DMA per sub-pair as its divides finish
                        t0 = n * 4 + sub - 1
                        nc.sync.dma_start(
                            out=out_h[t0 * 128 : (t0 + 2) * 128, :].rearrange(
                                "(t p) i -> p t i", p=128
                            ),
                            in_=out_sb[:, t0 : t0 + 2, :],
                        )
                if n != NQB - 1:
                    qs = slice(n * QB, (n + 1) * QB)
                    nc.sync.dma_start(
                        out=out_h[qs, :].rearrange("(t p) i -> p t i", p=128),
                        in_=out_sb[:, n * 4 : (n + 1) * 4, :],
                    )

            # ---- software-pipelined emission, paced by chunk DMA arrival:
            # all four QK projections (+fp8 folds) first with score pairs
            # emitted as their kf/qf chunks complete, V projections deferred
            # (only needed by the AV stage), AV blocks at pt-pool turnover
            # points. reps > 1 repeats the whole computation for benchmarking.
            for rep in range(reps):
                dma_embT_chunk(0)
                if rep == 0:
                    nc.sync.dma_start(out=bias_sb[:], in_=bias_h[:])
                    nc.sync.dma_start(
                        out=wv_sb[:],
                        in_=wv_h[:].rearrange("p (c w) -> p c w", c=6),
                    )
                for n in range(1, NQB):
                    dma_embT_chunk(n)
                if rep == 0 and do_attn:
                    # PE warm-up during the DMA lead-in: dummy matmuls ramp the
                    # HAM clock gate (1.2 -> 2.4 GHz) and keep the PE busy
                    # until the first embT half lands.
                    wmm = psS.tile([128, 1024], F32, tag="sc", name="warmmm")
                    for i in range(14):
                        nc.tensor.matmul(
                            wmm[:, (i % 2) * 512 : (i % 2) * 512 + 128],
                            lhsT=ident_bf[:, 0:128],
                            rhs=ident_bf[:, 0:128],
                            start=True,
                            stop=True,
                        )
                if do_proj:
                    proj_qk_chunk(0)
                if do_attn:
                    attn_pair(0, 0)
                    attn_pair(0, 1)
                if do_proj:
                    proj_qk_chunk(1)
                if do_attn:
                    attn_pair(0, 2)
                    attn_pair(0, 3)
                    for j in range(4):
                        attn_pair(1, j)
                if do_proj:
                    proj_qk_chunk(2)
                if do_attn:
                    attn_pair(0, 4)
                    attn_pair(0, 5)
                    attn_pair(1, 4)
                    attn_pair(1, 5)
                if do_proj:
                    proj_qk_chunk(3)
                if do_attn:
                    attn_pair(0, 6)
                    attn_pair(0, 7)
                    attn_pair(1, 6)
                    attn_pair(1, 7)
                if do_proj:
                    proj_v_chunk(0)
                    proj_v_chunk(1)
                if do_attn:
                    for j in range(4):
                        attn_pair(2, j)
                if do_proj:
                    proj_v_chunk(2)
                    proj_v_chunk(3)
                if do_attn:
                    for j in range(4, NKP):
                        attn_pair(2, j)
                    av_block(0)
                    if do_out:
                        out_block(0)
                    for j in range(4):
                        attn_pair(3, j)
                    av_block(1)
                    if do_out:
                        out_block(1)
                    for j in range(4, NKP):
                        attn_pair(3, j)
                    av_block(2)
                    if do_out:
                        out_block(2)
                    av_block(3)
                    if do_out:
                        out_block(3)
            if not do_out:
                nc.gpsimd.memset(out_sb[:, 0:1, :], 0.0)
                nc.sync.dma_start(
                    out=out_h[:].rearrange("(t p) i -> p t i", p=128),
                    in_=out_sb[:],
                )

    split_multi_waits(nc)
    return nc


_NC_CACHE = None


def _get_nc():
    global _NC_CACHE
    if _NC_CACHE is None:
        _NC_CACHE = build_nc()
    return _NC_CACHE


def make_in_maps(emb_input, Wq, bq, Wk, bk, Wv, bv):
    bf16 = ml_dtypes.bfloat16
    WqT = np.ascontiguousarray(Wq.T).astype(bf16)  # (768, 64)
    WkT = np.ascontiguousarray(Wk.T).astype(bf16)
    WvT = np.ascontiguousarray(Wv.T).astype(bf16)
    # partition-major: (6, 128, w) -> (128, 6*w) contiguous per partition
    wqk = np.concatenate([WqT, WkT], axis=1)  # (768, 128)
    wqk = np.ascontiguousarray(
        wqk.reshape(6, 128, 128).transpose(1, 0, 2).reshape(128, 768)
    )
    wv = np.ascontiguousarray(
        WvT.reshape(6, 128, 64).transpose(1, 0, 2).reshape(128, 384)
    )
    biases = np.zeros((128, 2), np.float32)
    biases[0:64, 0] = bq
    biases[0:64, 1] = bv
    in_maps = []
    for i in range(NCORES):
        embT = np.ascontiguousarray(emb_input[i].T).astype(bf16)  # (768, 2048)
        in_maps.append({"embT": embT, "wqk": wqk, "wv": wv, "biases": biases})
    return in_maps


def run(emb_input, Wq, bq, Wk, bk, Wv, bv, trace=False):
    nc = _get_nc()
    in_maps = make_in_maps(emb_input, Wq, bq, Wk, bk, Wv, bv)
    res = run_bass_kernel_spmd(nc, in_maps, core_ids=list(range(NCORES)), trace=trace)
    out = np.stack([res.results[i]["out"] for i in range(NCORES)], axis=0)
    return out.astype(np.float32), res


def kernel(emb_input, Wq, bq, Wk, bk, Wv, bv):
    out, _ = run(emb_input, Wq, bq, Wk, bk, Wv, bv, trace=False)
    return out
